# revision 35
# baseline (speedup 1.0000x reference)
"""Trainium2 Bass kernel for nn_DenseAttentionLayer (gnn_message_passing).

Math (reference):
    in_fts = context @ W_common.T            # (N, HID)
    left   = in_fts @ w_left + b_left        # (N,)
    right  = in_fts @ w_right + b_right      # (N,)
    logits = leaky_relu(left[:,None] + right[None,:], 0.2)
    logits = where(adj <= 0, -inf, logits)
    coefs  = softmax(logits, axis=-1)
    out    = relu(coefs @ relation)          # (N, REL_DIM)

Key design (v2 - transposed layout):
  * left = context @ (W_common.T @ w_left) + b_left (host-folded weights).
  * Layout: partition dim = j (columns of the NxN logits), free dim = i
    (the core's own rows).  adj is host-transposed per core into
    adjm[j, i] = 512*(adj-1) in fp16 ({0, -512}).  Then
        u = left_i + right_j + adjm
    is the exact logits for unmasked entries and <= -500 for masked
    ones, so exp underflows masked entries to exactly 0 - no separate
    mask multiply and no row-max pass (logits are O(1)).
  * exp(leaky(x)) = max(exp(u), exp(0.2u)) (exp monotone; HW Lrelu LUT
    has fixed 0.01 slope).  Two variants, mixed per-group to balance
    DVE vs ACT load:
      - 2-exp: e1=Exp(u), e2=Exp(0.2u) on ACT; zm=max(e1,e2) on DVE.
      - 1-exp: a=0.2u (TS), t=max(u,a) (TT) on DVE; zm=Exp(t) on ACT.
  * DVE fast modes: tensor_scalar runs 4x / tensor_tensor 2x when all
    non-scalar operands are 2-byte dtypes in SBUF; scalar_tensor_tensor
    is always 1x -> chain built from TS/TT in fp16.
  * The u-add (u += left_bcast) optionally runs on the otherwise-idle
    GPSIMD (Pool) engine for some groups (cfg pool_add).
  * left/right dot products (ctx @ v) run on the PE from host-transposed
    ctxT tiles, accumulating into the spare region of PSUM bank 7 which
    is shared with acc7: a matmul with start=True zeroes the whole 2KB
    bank, so only the very first dots matmul of each rep uses start=True
    and every other shared-bank matmul accumulates (skip_group_check).
    The TS mask op then reads right_j directly from PSUM as its
    per-partition scalar.
  * zm tiles [j, i] are directly matmul lhsT: acc[i,:] += zm.T @
    rel_aug[j,:] - no PE transposes, no PSUM evacuations.  Softmax
    denominator comes free as column 256 of the matmul (ones column).
  * finalize: reciprocal (DVE) + Relu*scale (ACT) + DMA out.

Sharding (8 cores): row-shard the N x N logits; each core owns r = N/8
rows (i), sees all N columns (j).  All params + rel + ctx replicated.
"""

import os
import sys

for _p in ("/opt/trn_rl_repo",):
    if _p not in sys.path and os.path.isdir(_p):
        sys.path.insert(0, _p)

from contextlib import ExitStack

import ml_dtypes
import numpy as np

# ---------------------------------------------------------------- constants
N = 8192  # num relations
IN = 512  # 2 * entity dim (context feature dim)
D = 256  # relation dim (output dim)
NCORES = 8
P = 128
MASKB = 512.0  # mask offset: adjm = MASKB*(adj-1), masked -> -512

_CACHE = {}


# ------------------------------------------------------------------ builder
def build_program(cfg):
    """Build the SPMD single-core Bass program."""
    import concourse.bass as bass
    import concourse.tile as tile
    from concourse import bacc, mybir

    f32 = mybir.dt.float32
    bf16 = mybir.dt.bfloat16
    f16 = mybir.dt.float16

    n = cfg["n"]  # full N (j extent)
    r = cfg["r"]  # rows per core (i extent)
    g = cfg["g"]  # j-tiles per group
    reps = cfg.get("reps", 1)  # >1: loop whole kernel (timing harness only)
    n1exp = cfg.get("n1exp", 2)  # of ng groups, how many use 1-exp variant
    pool_add = cfg.get("pool_add", 10)  # of ng groups: u-add on gpsimd
    dots = cfg.get("dots", "pe")  # 'pe' | 'dve'
    pref = cfg.get("pref", 2)  # dots group lookahead
    use_prelu = cfg.get("use_prelu", True)  # parametric_relu honors alpha

    ni = r // P  # i-blocks per core (8)
    njt = n // P  # j-tiles (64)
    ng = njt // g  # groups
    nk = IN // P  # k-tiles (4)

    assert ni == 8
    # shared PSUM bank 7 layout (f32 cols): acc7 [0:257], right dots
    # [257:257+njt], left dots [257+njt : 257+njt+ni]
    RD0 = D + 1
    LD0 = RD0 + njt
    assert LD0 + ni <= 512

    nc = bacc.Bacc("TRN2", target_bir_lowering=False, debug=False)

    f8 = mybir.dt.float8e5
    adjm = nc.dram_tensor("adjm", [n, r], f8, kind="ExternalInput")
    ctx_own = nc.dram_tensor("ctx_own", [r, IN], bf16, kind="ExternalInput")
    if cfg.get("dots", "pe") == "dve":
        ctx_dve = nc.dram_tensor("ctx_dve", [n, IN], bf16, kind="ExternalInput")
    ctxT = nc.dram_tensor("ctxT", [IN, n], bf16, kind="ExternalInput")
    rel_in = nc.dram_tensor("rel_in", [n, D], f16, kind="ExternalInput")
    vl_in = nc.dram_tensor("vl_in", [IN], bf16, kind="ExternalInput")
    vr_in = nc.dram_tensor("vr_in", [IN], bf16, kind="ExternalInput")
    lbias = nc.dram_tensor("lbias", [1], f32, kind="ExternalInput")  # b_l+b_r
    out = nc.dram_tensor("out", [r, D], f16, kind="ExternalOutput")
    l_scr = nc.dram_tensor("l_scr", [r], f16)  # left bounce scratch
    use_ag = cfg.get("use_ag", False)
    if use_ag:
        r_shard = nc.dram_tensor("r_shard", [r], f32)
        r_all = nc.dram_tensor("r_all", [n], f32, addr_space="Shared")

    with tile.TileContext(nc) as tc, ExitStack() as ctx:
        singles = ctx.enter_context(tc.tile_pool(name="singles", bufs=1))
        ctxT_pool = ctx.enter_context(tc.tile_pool(name="ctxTp", bufs=2))
        adj_pool = ctx.enter_context(tc.tile_pool(name="adjp", bufs=cfg.get("adj_bufs", 3)))
        u_pool = ctx.enter_context(tc.tile_pool(name="up", bufs=3))
        e1_pool = ctx.enter_context(tc.tile_pool(name="e1p", bufs=cfg.get("e_bufs", 4)))
        e2_pool = ctx.enter_context(tc.tile_pool(name="e2p", bufs=cfg.get("e_bufs", 4)))
        zm_pool = ctx.enter_context(tc.tile_pool(name="zmp", bufs=3))
        out_pool = ctx.enter_context(tc.tile_pool(name="outp", bufs=2))
        sm_pool = ctx.enter_context(tc.tile_pool(name="smp", bufs=2))
        acc_psum = ctx.enter_context(
            tc.tile_pool(name="accps", bufs=1, space="PSUM")
        )
        scr_pool = ctx.enter_context(tc.tile_pool(name="scrp", bufs=1))

        def _emit_body():
            # group spec: (start j-tile, width); smaller groups at the head
            # (faster ramp) and tail (shorter pipeline drain)
            gspecs = []
            pos = 0
            head = cfg.get("head_split", [2, 2])
            tail = cfg.get("tail_split", [2, 1, 1])
            for w in head:
                gspecs.append((pos, w)); pos += w
            while pos < njt - sum(tail):
                gspecs.append((pos, g)); pos += g
            for w in tail:
                gspecs.append((pos, w)); pos += w
            assert pos == njt
            ngr = len(gspecs)

            adj_tiles = {}

            adj_eng = nc.scalar if cfg.get("adj_queue", "act") == "act" else nc.sync

            def emit_adjm_dma(gi):
                # adjm DRAM layout is [p, jt, i] so each partition's group
                # slab is one contiguous run (1 DMA descriptor/partition)
                j0, gw = gspecs[gi]
                adjt = adj_pool.tile([P, gw, r], f8, tag="adj")
                adj_eng.dma_start(
                    out=adjt,
                    in_=bass.AP(
                        tensor=adjm,
                        offset=j0 * r,
                        ap=[[njt * r, P], [r, gw], [1, r]],
                    ),
                )
                adj_tiles[gi] = (adjt, gw)

            adj_pref = cfg.get("adj_pref", 2)
            for _gi in range(min(adj_pref, ngr)):
                emit_adjm_dma(_gi)

            # ---------------- phase 0: params ----------------
            vlb = singles.tile([P, nk], bf16, tag="vlb")
            nc.sync.dma_start(
                out=vlb, in_=bass.AP(tensor=vl_in, offset=0, ap=[[1, P], [P, nk]])
            )
            vrb = singles.tile([P, nk], bf16, tag="vrb")
            nc.sync.dma_start(
                out=vrb, in_=bass.AP(tensor=vr_in, offset=0, ap=[[1, P], [P, nk]])
            )
            lbb = singles.tile([P, 1], f32, tag="lbb")
            nc.sync.dma_start(
                out=lbb, in_=bass.AP(tensor=lbias, offset=0, ap=[[0, P], [1, 1]])
            )

            # relation chunks, each augmented with a ones column
            # (denominator trick).  Per-chunk tiles + deferred DMA keep the
            # big rel read off the critical path and the deps fine-grained.
            nch = cfg.get("dot_chunks", 8)
            tpc = njt // nch  # j-tiles per chunk
            rel_chunks = [
                singles.tile([P, tpc, D + 1], f16, name=f"relch{c}", tag=f"relch{c}")
                for c in range(nch)
            ]

            rel_eng = {"sp": nc.sync, "act": nc.scalar, "pool": nc.gpsimd}[
                cfg.get("rel_queue", "sp")
            ]

            def emit_rel_dma(c):
                nc.vector.memset(rel_chunks[c][:, :, D : D + 1], 1.0)
                rel_eng.dma_start(
                    out=rel_chunks[c][:, :, 0:D],
                    in_=bass.AP(
                        tensor=rel_in,
                        offset=c * tpc * P * D,
                        ap=[[D, P], [P * D, tpc], [1, D]],
                    ),
                )

            def rel_tile(jt):
                return rel_chunks[jt // tpc][:, jt % tpc, :]

            # PSUM accumulators: banks 0-6 own i-blocks 0-6; bank 7 shared
            # between acc7 and the left/right dot-product columns.
            accs = [
                acc_psum.tile([P, 512], f32, tag=f"acc{ib}", name=f"acc{ib}")
                for ib in range(ni - 1)
            ]
            shared = acc_psum.tile([P, 512], f32, tag="accsh", name="accsh")
            accs.append(shared)

            # ---------------- dot products (prologue) ----------------
            # left/right dots accumulate in the spare region of shared PSUM
            # bank 7 and are evacuated to SBUF in chunks.  Each chunk gets
            # its OWN SBUF tile: dependency tracking is tile-granular, so a
            # single shared tile would make the first main-loop read wait
            # for the LAST chunk's copy.
            left_sb = singles.tile([P, ni], f32, tag="left_sb")
            left_cols = left_sb[:, :]
            right_chunks = [
                singles.tile([P, tpc], f32, name=f"rchunk{c}", tag=f"rchunk{c}")
                for c in range(nch)
            ]

            def right_col(jt):
                return right_chunks[jt // tpc][:, jt % tpc : jt % tpc + 1]


            # left dots on DVE (STT accum) from natural-layout ctx_own:
            # DVE is idle at program start and this keeps the left_b chain
            # (lc16 -> DRAM bounce -> broadcast), which gates the first
            # u-add, off the PE/prologue critical path.
            vlb_f = singles.tile([P, IN], bf16, tag="vlb_f")
            nc.sync.dma_start(
                out=vlb_f,
                in_=bass.AP(tensor=vl_in, offset=0, ap=[[0, P], [1, IN]]),
            )
            cow = singles.tile([P, ni, IN], bf16, tag="cow")
            nc.sync.dma_start(
                out=cow,
                in_=bass.AP(
                    tensor=ctx_own, offset=0, ap=[[ni * IN, P], [1, ni * IN]]
                ),
            )
            for t in range(ni):
                scr = scr_pool.tile([P, IN], f32, tag="scr")
                nc.vector.scalar_tensor_tensor(
                    out=scr, in0=cow[:, t, :], scalar=0.0, in1=vlb_f,
                    op0=mybir.AluOpType.bypass, op1=mybir.AluOpType.mult,
                    accum_out=left_cols[:, t : t + 1],
                )

            if use_ag:
                # right dots for OWN rows only (from cow, like left), then
                # AllGather the 8192-float right vector (32KB) instead of
                # re-reading the full replicated ctxT (8.4MB per core)
                vrb_f = singles.tile([P, IN], bf16, tag="vrb_f")
                nc.sync.dma_start(
                    out=vrb_f,
                    in_=bass.AP(tensor=vr_in, offset=0, ap=[[0, P], [1, IN]]),
                )
                r_own = singles.tile([P, ni], f32, tag="r_own")
                for t in range(ni):
                    scr = scr_pool.tile([P, IN], f32, tag="scr")
                    nc.vector.scalar_tensor_tensor(
                        out=scr, in0=cow[:, t, :], scalar=0.0, in1=vrb_f,
                        op0=mybir.AluOpType.bypass, op1=mybir.AluOpType.mult,
                        accum_out=r_own[:, t : t + 1],
                    )
                nc.sync.dma_start(
                    out=bass.AP(tensor=r_shard, offset=0, ap=[[1, P], [P, ni]]),
                    in_=r_own,
                )
                nc.gpsimd.collective_compute(
                    "AllGather",
                    mybir.AluOpType.bypass,
                    replica_groups=[list(range(NCORES))],
                    ins=[r_shard[:]],
                    outs=[r_all[:]],
                )
                for c in range(nch):
                    nc.sync.dma_start(
                        out=right_chunks[c],
                        in_=bass.AP(
                            tensor=r_all,
                            offset=c * tpc * P,
                            ap=[[1, P], [P, tpc]],
                        ),
                    )
                    emit_rel_dma(c)
            elif dots == "pe":
                first_mm = [True]
                # right dots in chunks, each evacuated to SBUF as soon as
                # ready so the main loop starts after chunk 0 (not all 64)
                for ch_i in range(nch):
                    for tt in range(tpc):
                        jt = ch_i * tpc + tt
                        if tt == 0:
                            # ctxT DRAM layout is [p, chunk, kt, j']: one
                            # contiguous nk*tpc*P run per partition
                            cht = ctxT_pool.tile(
                                [P, nk, tpc * P], bf16, tag="ctxT"
                            )
                            nc.sync.dma_start(
                                out=cht,
                                in_=bass.AP(
                                    tensor=ctxT,
                                    offset=ch_i * nk * tpc * P,
                                    ap=[
                                        [nch * nk * tpc * P, P],
                                        [1, nk * tpc * P],
                                    ],
                                ),
                            )
                        for k in range(nk):
                            nc.tensor.matmul(
                                shared[:, RD0 + jt : RD0 + jt + 1],
                                lhsT=cht[:, k, tt * P : (tt + 1) * P],
                                rhs=vrb[:, k : k + 1],
                                start=first_mm[0],  # zeroes bank 7 once/rep
                                stop=False,
                                skip_group_check=True,
                            )
                            first_mm[0] = False
                    nc.vector.tensor_copy(
                        right_chunks[ch_i],
                        shared[:, RD0 + ch_i * tpc : RD0 + (ch_i + 1) * tpc],
                    )
                    emit_rel_dma(ch_i)
            else:
                # DVE STT dots from packed ctx_dve; acc bank 7 is then a
                # normal accumulator (no shared-bank trickery at all)
                vrb_f = singles.tile([P, IN], bf16, tag="vrb_f")
                nc.sync.dma_start(
                    out=vrb_f,
                    in_=bass.AP(tensor=vr_in, offset=0, ap=[[0, P], [1, IN]]),
                )
                for ch_i in range(nch):
                    cdt = ctxT_pool.tile([P, tpc, IN], bf16, tag="ctxT")
                    nc.sync.dma_start(
                        out=cdt,
                        in_=bass.AP(
                            tensor=ctx_dve,
                            offset=ch_i * tpc * IN,
                            ap=[[njt * IN, P], [1, tpc * IN]],
                        ),
                    )
                    for tt in range(tpc):
                        jt = ch_i * tpc + tt
                        scr = scr_pool.tile([P, IN], f32, tag="scr")
                        nc.vector.scalar_tensor_tensor(
                            out=scr, in0=cdt[:, tt, :], scalar=0.0, in1=vrb_f,
                            op0=mybir.AluOpType.bypass,
                            op1=mybir.AluOpType.mult,
                            accum_out=right_col(jt),
                        )
                    emit_rel_dma(ch_i)

            # left + (b_l + b_r) -> fp16, bounce via DRAM, broadcast back
            lc16 = singles.tile([P, ni], f16, tag="lc16")
            nc.vector.tensor_scalar(
                out=lc16, in0=left_cols, scalar1=lbb[:, 0:1], scalar2=None,
                op0=mybir.AluOpType.add,
            )
            # bounce + broadcast ride the Pool SWDGE queue: their sem waits
            # must not block the ACT sequencer (exps) or SP queue (bulk DMA)
            bq = cfg.get("bounce_queue", "act")
    
            bounce_eng = {"pool": nc.gpsimd, "act": nc.scalar, "sp": nc.sync}[bq]
            bounce_eng.dma_start(
                out=bass.AP(tensor=l_scr, offset=0, ap=[[1, P], [P, ni]]),
                in_=lc16,
            )
            left_b = singles.tile([P, g, r], f16, tag="left_b")
            for t in range(g):
                bounce_eng.dma_start(
                    out=left_b[:, t, :],
                    in_=bass.AP(tensor=l_scr, offset=0, ap=[[0, P], [1, r]]),
                )

            # ------------------------- main loop ----------------------------
            # which groups use the 1-exp variant / pool u-add (spread evenly)
            n_1exp_done = 0
            n_pool_done = 0
            pending = []  # software pipeline: consume stage delayed 1 group

            def consume(ent):
                # 2-exp groups: the DVE max is emitted here, one iteration
                # after its exps, so DVE's in-order queue never stalls on ACT
                j0, gw, kind, tiles = ent
                if kind == "2exp":
                    e1t, e2t, zmt = tiles
                    nc.vector.tensor_max(zmt, e1t, e2t)
                else:
                    zmt = tiles[0]
                for tt in range(gw):
                    jt = j0 + tt
                    for ib in range(ni):
                        nc.tensor.matmul(
                            accs[ib][:, 0 : D + 1],
                            lhsT=zmt[:, tt, ib * P : (ib + 1) * P],
                            rhs=rel_tile(jt),
                            start=(jt == 0 and (ib < ni - 1 or dots != "pe")),
                            stop=(jt == njt - 1),
                            skip_group_check=(ib == ni - 1 and dots == "pe"),
                        )

            for gi in range(ngr):
                if gi + adj_pref < ngr:
                    emit_adjm_dma(gi + adj_pref)
                adjt, gw = adj_tiles.pop(gi)
                j0 = gspecs[gi][0]
                # u = (adjm + right_j) + left_i fused in one STT pass
                ut = u_pool.tile([P, gw, r], f16, tag="u")
                for tt in range(gw):
                    jt = j0 + tt
                    nc.vector.scalar_tensor_tensor(
                        out=ut[:, tt, :], in0=adjt[:, tt, :],
                        scalar=right_col(jt), in1=left_b[:, tt, :],
                        op0=mybir.AluOpType.add, op1=mybir.AluOpType.add,
                    )

                use_1exp = (n1exp * (gi + 1)) // ngr > n_1exp_done
                zmt = zm_pool.tile([P, gw, r], f16, tag="zm")
                if use_1exp:
                    # DVE-heavy: leaky via TS(0.2u) + max, single ACT exp
                    n_1exp_done += 1
                    at = e1_pool.tile([P, gw, r], f16, tag="e1")
                    nc.vector.tensor_scalar(
                        out=at, in0=ut, scalar1=0.2, scalar2=None,
                        op0=mybir.AluOpType.mult,
                    )
                    tt_ = e2_pool.tile([P, gw, r], f16, tag="e2")
                    nc.vector.tensor_max(tt_, ut, at)
                    nc.scalar.activation(
                        zmt, tt_, mybir.ActivationFunctionType.Exp,
                        bias=0.0, scale=1.0,
                    )
                    pending.append((j0, gw, "1exp", (zmt,)))
                elif use_prelu:
                    # ACT-only: parametric_relu (alpha=0.2) + exp, both in
                    # the exp table set -> no table reload, no DVE max
                    t16 = e1_pool.tile([P, gw, r], f16, tag="e1")
                    nc.scalar.activation(
                        t16, ut, mybir.ActivationFunctionType.Prelu,
                        bias=0.0, scale=1.0, alpha=0.2,
                    )
                    nc.scalar.activation(
                        zmt, t16, mybir.ActivationFunctionType.Exp,
                        bias=0.0, scale=1.0,
                    )
                    pending.append((j0, gw, "1exp", (zmt,)))
                else:
                    e1t = e1_pool.tile([P, gw, r], f16, tag="e1")
                    nc.scalar.activation(
                        e1t, ut, mybir.ActivationFunctionType.Exp,
                        bias=0.0, scale=1.0,
                    )
                    e2t = e2_pool.tile([P, gw, r], f16, tag="e2")
                    nc.scalar.activation(
                        e2t, ut, mybir.ActivationFunctionType.Exp,
                        bias=0.0, scale=0.2,
                    )
                    pending.append((j0, gw, "2exp", (e1t, e2t, zmt)))

                if len(pending) > 1:
                    consume(pending.pop(0))
            while pending:
                consume(pending.pop(0))

            # ------------------------ finalize ------------------------------
            # finalize entirely on DVE: relu(num/den) = max(num*recip, 0)
            # (recip > 0), avoiding ACT table swaps and engine ping-pong
            for ib in range(ni):
                recip = sm_pool.tile([P, 1], f32, tag="recip")
                nc.vector.reciprocal(recip, accs[ib][:, D : D + 1])
                ob = out_pool.tile([P, D], f16, tag="ob")
                nc.vector.tensor_scalar(
                    out=ob, in0=accs[ib][:, 0:D], scalar1=recip[:, 0:1],
                    scalar2=0.0, op0=mybir.AluOpType.mult,
                    op1=mybir.AluOpType.max,
                )
                nc.sync.dma_start(out=out[ib * P : (ib + 1) * P, :], in_=ob)

        flat_reps = cfg.get("flat_reps", 1)  # sim-only: unrolled reps
        if reps > 1:
            with tc.For_i(0, reps, 1):
                _emit_body()
        else:
            for _ in range(flat_reps):
                _emit_body()

    nc.compile()
    return nc


_BASE_CFG = dict(n=N, r=N // NCORES, g=4, n1exp=8, pool_add=0, dots="pe",
                 pref=2, use_prelu=True, dot_chunks=8, head_split=[2, 2],
                 tail_split=[2, 1, 1], e_bufs=4)


def _get_program(cfg_key):
    if cfg_key not in _CACHE:
        _CACHE[cfg_key] = build_program(dict(_BASE_CFG))
    return _CACHE[cfg_key]


def prepare_in_maps(relation, context, adj_tensor, W_common, w_left, b_left,
                    w_right, b_right):
    relation = np.asarray(relation, dtype=np.float32)
    context = np.asarray(context, dtype=np.float32)
    adj_tensor = np.asarray(adj_tensor, dtype=np.float32)
    W_common = np.asarray(W_common, dtype=np.float32)
    w_left = np.asarray(w_left, dtype=np.float32)
    w_right = np.asarray(w_right, dtype=np.float32)
    b_l = float(np.asarray(b_left))
    b_r = float(np.asarray(b_right))

    # host-side parameter folding (weights only, no activations)
    v_left = (W_common.T @ w_left).astype(ml_dtypes.bfloat16)
    v_right = (W_common.T @ w_right).astype(ml_dtypes.bfloat16)
    lbias = np.array([b_l + b_r], dtype=np.float32)

    relh = relation.astype(np.float16)
    ctxb = context.astype(ml_dtypes.bfloat16)
    # ctxT packed as [p, chunk, kt, j']: partition-contiguous chunk reads
    nch = _BASE_CFG.get("dot_chunks", 8)
    tpcP = N // nch  # j per chunk
    ctxT8 = np.ascontiguousarray(
        ctxb.T.reshape(4, P, nch, tpcP).transpose(1, 2, 0, 3)
    )  # [128, nch, 4, tpcP]

    # adjm[j, i] = MASKB * (adj[i, j] - 1):  0 if edge, -512 if masked
    adjm_full = ((adj_tensor - 1.0) * MASKB).astype(ml_dtypes.float8_e5m2)

    rows = N // NCORES
    njt = N // P
    in_maps = []
    for c in range(NCORES):
        sl = slice(c * rows, (c + 1) * rows)
        # adjm slab [j, i] -> [p, jt, i] (partition-contiguous group reads)
        aT = adjm_full[sl].T.reshape(njt, P, rows)
        m = {
            "adjm": np.ascontiguousarray(aT.transpose(1, 0, 2)),
            "ctx_own": np.ascontiguousarray(
                ctxb[sl].reshape(rows // P, P, IN).transpose(1, 0, 2)
            ),
            "ctxT": ctxT8,
            "rel_in": relh,
            "vl_in": v_left,
            "vr_in": v_right,
            "lbias": lbias,
        }
        in_maps.append(m)
    return in_maps


# ------------------------------------------------------------------- entry
def kernel(relation, context, adj_tensor, W_common, w_left, b_left, w_right,
           b_right):
    from concourse.bass_utils import run_bass_kernel_spmd

    in_maps = prepare_in_maps(relation, context, adj_tensor, W_common,
                              w_left, b_left, w_right, b_right)
    nc = _get_program("main")
    last_err = None
    for _attempt in range(3):
        try:
            res = run_bass_kernel_spmd(nc, in_maps, list(range(NCORES)))
            outs = [np.asarray(res.results[c]["out"]) for c in range(NCORES)]
            return np.concatenate(outs, axis=0).astype(np.float32)
        except Exception as e:  # transient device-unrecoverable seen on axon
            last_err = e
            import time as _time

            try:
                import jax

                jax.clear_caches()
            except Exception:
                pass
            _time.sleep(3.0)
    raise last_err


# revision 36
# speedup vs baseline: 1.3947x; 1.3947x over previous
"""Trainium2 Bass kernel for nn_DenseAttentionLayer (gnn_message_passing).

Math (reference):
    in_fts = context @ W_common.T            # (N, HID)
    left   = in_fts @ w_left + b_left        # (N,)
    right  = in_fts @ w_right + b_right      # (N,)
    logits = leaky_relu(left[:,None] + right[None,:], 0.2)
    logits = where(adj <= 0, -inf, logits)
    coefs  = softmax(logits, axis=-1)
    out    = relu(coefs @ relation)          # (N, REL_DIM)

Design (v5: transposed layout, fp8 mask, Prelu):
  * left = context @ (W_common.T @ w_left) + b_left (host-folded weights).
  * Layout: partition dim = j (columns of the NxN logits), free dim = i
    (the core's own rows).  adj is host-transformed per core into
    adjm[j, i] = 512*(adj-1) in fp8_e5m2 ({0, -512} exactly), packed
    [p, jt, i] so each partition's group slab is one contiguous DMA run.
    Then u = left_i + right_j + adjm is the exact logits for unmasked
    entries and <= -500 for masked ones, so exp underflows masked
    entries to exactly 0 - no separate mask multiply, no row-max pass
    (logits are O(1)), and the mask stream is 1 byte/element.  This
    kernel is strongly DMA-bound on the axon TRN2 target, so mask bytes
    dominate the runtime.
  * u is built in ONE scalar_tensor_tensor pass (adjm + right_j scalar +
    left_i broadcast).  STT always runs 1x on DVE (no 2-byte fast mode),
    which the fp8 operand would forfeit anyway - so the fuse is free.
  * exp(leaky(x)): two per-group variants balance ACT vs DVE load:
      - prelu: t = Prelu(u, alpha=0.2); zm = Exp(t).  parametric_relu
        honors alpha on this HW and shares the exp activation table
        (no table reloads).  All on ACT.
      - 1exp: a = 0.2u (TS 4x), t = max(u,a) (TT 2x) on DVE; Exp on ACT.
  * zm tiles [j, i] in fp16 are directly the matmul lhsT:
    acc[i,:] += zm.T @ rel_aug[j,:] - no PE transposes, no PSUM
    evacuations.  Softmax denominator comes free as column 256 of the
    matmul (ones column on rel_aug).  8 PSUM banks = 8 accumulators.
  * left/right dot products (ctx @ v) run on the PE from host-transposed
    ctxT tiles into the spare region of PSUM bank 7 (shared with acc7:
    matmul start=True zeroes the whole 2KB bank, so exactly one dots
    matmul per rep starts; everything else accumulates with
    skip_group_check).  Dots are evacuated to per-chunk SBUF tiles
    (dep tracking is tile-granular) and the mask STT reads right_j from
    SBUF as its per-partition scalar.
  * DMA queue split: adjm + left-bounce ride the ACT HWDGE queue,
    ctxT/rel/ctx_own the SP queue (a slow Pool-SWDGE experiment lost).
  * Emission is software-pipelined (consume stage delayed one group) so
    DVE's in-order queue never stalls on ACT output; group widths taper
    at head/tail ([2,2]...[2,1,1]) for faster ramp and drain.
  * finalize entirely on DVE: relu(num/den) = max(num*recip, 0), written
    out as fp16 (upconverted on host).

Sharding (8 cores): row-shard the N x N logits; each core owns r = N/8
rows (i), sees all N columns (j).  All params + rel + ctx replicated.
Measured on axon TRN2: ~165 us/iter (baseline: 312-484 us), rel err
2.9e-3 (tolerance 2e-2).
"""

import os
import sys

for _p in ("/opt/trn_rl_repo",):
    if _p not in sys.path and os.path.isdir(_p):
        sys.path.insert(0, _p)

from contextlib import ExitStack

import ml_dtypes
import numpy as np

# ---------------------------------------------------------------- constants
N = 8192  # num relations
IN = 512  # 2 * entity dim (context feature dim)
D = 256  # relation dim (output dim)
NCORES = 8
P = 128
MASKB = 512.0  # mask offset: adjm = MASKB*(adj-1), masked -> -512

_CACHE = {}


# ------------------------------------------------------------------ builder
def build_program(cfg):
    """Build the SPMD single-core Bass program."""
    import concourse.bass as bass
    import concourse.tile as tile
    from concourse import bacc, mybir

    f32 = mybir.dt.float32
    bf16 = mybir.dt.bfloat16
    f16 = mybir.dt.float16

    n = cfg["n"]  # full N (j extent)
    r = cfg["r"]  # rows per core (i extent)
    g = cfg["g"]  # j-tiles per group
    reps = cfg.get("reps", 1)  # >1: loop whole kernel (timing harness only)
    n1exp = cfg.get("n1exp", 2)  # of ng groups, how many use 1-exp variant
    pool_add = cfg.get("pool_add", 10)  # of ng groups: u-add on gpsimd
    dots = cfg.get("dots", "pe")  # 'pe' | 'dve'
    pref = cfg.get("pref", 2)  # dots group lookahead
    use_prelu = cfg.get("use_prelu", True)  # parametric_relu honors alpha

    ni = r // P  # i-blocks per core (8)
    njt = n // P  # j-tiles (64)
    ng = njt // g  # groups
    nk = IN // P  # k-tiles (4)

    assert ni == 8
    # shared PSUM bank 7 layout (f32 cols): acc7 [0:257], right dots
    # [257:257+njt], left dots [257+njt : 257+njt+ni]
    RD0 = D + 1
    LD0 = RD0 + njt
    assert LD0 + ni <= 512

    nc = bacc.Bacc("TRN2", target_bir_lowering=False, debug=False)

    f8 = mybir.dt.float8e5
    adjm = nc.dram_tensor("adjm", [n, r], f8, kind="ExternalInput")
    ctx_own = nc.dram_tensor("ctx_own", [r, IN], bf16, kind="ExternalInput")
    if cfg.get("dots", "pe") == "dve":
        ctx_dve = nc.dram_tensor("ctx_dve", [n, IN], bf16, kind="ExternalInput")
    ctxT = nc.dram_tensor("ctxT", [IN, n], bf16, kind="ExternalInput")
    rel_in = nc.dram_tensor("rel_in", [n, D], f16, kind="ExternalInput")
    vl_in = nc.dram_tensor("vl_in", [IN], bf16, kind="ExternalInput")
    vr_in = nc.dram_tensor("vr_in", [IN], bf16, kind="ExternalInput")
    lbias = nc.dram_tensor("lbias", [1], f32, kind="ExternalInput")  # b_l+b_r
    out = nc.dram_tensor("out", [r, D], f16, kind="ExternalOutput")
    l_scr = nc.dram_tensor("l_scr", [r], f16)  # left bounce scratch
    use_ag = cfg.get("use_ag", False)
    if use_ag:
        r_shard = nc.dram_tensor("r_shard", [r], f32)
        r_all = nc.dram_tensor("r_all", [n], f32, addr_space="Shared")

    with tile.TileContext(nc) as tc, ExitStack() as ctx:
        singles = ctx.enter_context(tc.tile_pool(name="singles", bufs=1))
        ctxT_pool = ctx.enter_context(tc.tile_pool(name="ctxTp", bufs=2))
        adj_pool = ctx.enter_context(tc.tile_pool(name="adjp", bufs=cfg.get("adj_bufs", 3)))
        u_pool = ctx.enter_context(tc.tile_pool(name="up", bufs=3))
        e1_pool = ctx.enter_context(tc.tile_pool(name="e1p", bufs=cfg.get("e_bufs", 4)))
        e2_pool = ctx.enter_context(tc.tile_pool(name="e2p", bufs=cfg.get("e_bufs", 4)))
        zm_pool = ctx.enter_context(tc.tile_pool(name="zmp", bufs=3))
        out_pool = ctx.enter_context(tc.tile_pool(name="outp", bufs=2))
        sm_pool = ctx.enter_context(tc.tile_pool(name="smp", bufs=2))
        acc_psum = ctx.enter_context(
            tc.tile_pool(name="accps", bufs=1, space="PSUM")
        )
        scr_pool = ctx.enter_context(tc.tile_pool(name="scrp", bufs=1))

        def _emit_body():
            # group spec: (start j-tile, width); smaller groups at the head
            # (faster ramp) and tail (shorter pipeline drain)
            gspecs = []
            pos = 0
            head = cfg.get("head_split", [2, 2])
            tail = cfg.get("tail_split", [2, 1, 1])
            for w in head:
                gspecs.append((pos, w)); pos += w
            while pos < njt - sum(tail):
                gspecs.append((pos, g)); pos += g
            for w in tail:
                gspecs.append((pos, w)); pos += w
            assert pos == njt
            ngr = len(gspecs)

            adj_tiles = {}

            adj_eng = nc.scalar if cfg.get("adj_queue", "act") == "act" else nc.sync

            def emit_adjm_dma(gi):
                # adjm DRAM layout is [p, jt, i] so each partition's group
                # slab is one contiguous run (1 DMA descriptor/partition)
                j0, gw = gspecs[gi]
                adjt = adj_pool.tile([P, gw, r], f8, tag="adj")
                adj_eng.dma_start(
                    out=adjt,
                    in_=bass.AP(
                        tensor=adjm,
                        offset=j0 * r,
                        ap=[[njt * r, P], [r, gw], [1, r]],
                    ),
                )
                adj_tiles[gi] = (adjt, gw)

            adj_pref = cfg.get("adj_pref", 2)
            for _gi in range(min(adj_pref, ngr)):
                emit_adjm_dma(_gi)

            # ---------------- phase 0: params ----------------
            vlb = singles.tile([P, nk], bf16, tag="vlb")
            nc.sync.dma_start(
                out=vlb, in_=bass.AP(tensor=vl_in, offset=0, ap=[[1, P], [P, nk]])
            )
            vrb = singles.tile([P, nk], bf16, tag="vrb")
            nc.sync.dma_start(
                out=vrb, in_=bass.AP(tensor=vr_in, offset=0, ap=[[1, P], [P, nk]])
            )
            lbb = singles.tile([P, 1], f32, tag="lbb")
            nc.sync.dma_start(
                out=lbb, in_=bass.AP(tensor=lbias, offset=0, ap=[[0, P], [1, 1]])
            )

            # relation chunks, each augmented with a ones column
            # (denominator trick).  Per-chunk tiles + deferred DMA keep the
            # big rel read off the critical path and the deps fine-grained.
            nch = cfg.get("dot_chunks", 8)
            tpc = njt // nch  # j-tiles per chunk
            rel_chunks = [
                singles.tile([P, tpc, D + 1], f16, name=f"relch{c}", tag=f"relch{c}")
                for c in range(nch)
            ]

            rel_eng = {"sp": nc.sync, "act": nc.scalar, "pool": nc.gpsimd}[
                cfg.get("rel_queue", "sp")
            ]

            def emit_rel_dma(c):
                nc.vector.memset(rel_chunks[c][:, :, D : D + 1], 1.0)
                rel_eng.dma_start(
                    out=rel_chunks[c][:, :, 0:D],
                    in_=bass.AP(
                        tensor=rel_in,
                        offset=c * tpc * P * D,
                        ap=[[D, P], [P * D, tpc], [1, D]],
                    ),
                )

            def rel_tile(jt):
                return rel_chunks[jt // tpc][:, jt % tpc, :]

            # PSUM accumulators: banks 0-6 own i-blocks 0-6; bank 7 shared
            # between acc7 and the left/right dot-product columns.
            accs = [
                acc_psum.tile([P, 512], f32, tag=f"acc{ib}", name=f"acc{ib}")
                for ib in range(ni - 1)
            ]
            shared = acc_psum.tile([P, 512], f32, tag="accsh", name="accsh")
            accs.append(shared)

            # ---------------- dot products (prologue) ----------------
            # left/right dots accumulate in the spare region of shared PSUM
            # bank 7 and are evacuated to SBUF in chunks.  Each chunk gets
            # its OWN SBUF tile: dependency tracking is tile-granular, so a
            # single shared tile would make the first main-loop read wait
            # for the LAST chunk's copy.
            left_sb = singles.tile([P, ni], f32, tag="left_sb")
            left_cols = left_sb[:, :]
            right_chunks = [
                singles.tile([P, tpc], f32, name=f"rchunk{c}", tag=f"rchunk{c}")
                for c in range(nch)
            ]

            def right_col(jt):
                return right_chunks[jt // tpc][:, jt % tpc : jt % tpc + 1]


            # left dots on DVE (STT accum) from natural-layout ctx_own:
            # DVE is idle at program start and this keeps the left_b chain
            # (lc16 -> DRAM bounce -> broadcast), which gates the first
            # u-add, off the PE/prologue critical path.
            vlb_f = singles.tile([P, IN], bf16, tag="vlb_f")
            nc.sync.dma_start(
                out=vlb_f,
                in_=bass.AP(tensor=vl_in, offset=0, ap=[[0, P], [1, IN]]),
            )
            cow = singles.tile([P, ni, IN], bf16, tag="cow")
            nc.sync.dma_start(
                out=cow,
                in_=bass.AP(
                    tensor=ctx_own, offset=0, ap=[[ni * IN, P], [1, ni * IN]]
                ),
            )
            for t in range(ni):
                scr = scr_pool.tile([P, IN], f32, tag="scr")
                nc.vector.scalar_tensor_tensor(
                    out=scr, in0=cow[:, t, :], scalar=0.0, in1=vlb_f,
                    op0=mybir.AluOpType.bypass, op1=mybir.AluOpType.mult,
                    accum_out=left_cols[:, t : t + 1],
                )

            if use_ag:
                # right dots for OWN rows only (from cow, like left), then
                # AllGather the 8192-float right vector (32KB) instead of
                # re-reading the full replicated ctxT (8.4MB per core)
                vrb_f = singles.tile([P, IN], bf16, tag="vrb_f")
                nc.sync.dma_start(
                    out=vrb_f,
                    in_=bass.AP(tensor=vr_in, offset=0, ap=[[0, P], [1, IN]]),
                )
                r_own = singles.tile([P, ni], f32, tag="r_own")
                for t in range(ni):
                    scr = scr_pool.tile([P, IN], f32, tag="scr")
                    nc.vector.scalar_tensor_tensor(
                        out=scr, in0=cow[:, t, :], scalar=0.0, in1=vrb_f,
                        op0=mybir.AluOpType.bypass, op1=mybir.AluOpType.mult,
                        accum_out=r_own[:, t : t + 1],
                    )
                nc.sync.dma_start(
                    out=bass.AP(tensor=r_shard, offset=0, ap=[[1, P], [P, ni]]),
                    in_=r_own,
                )
                nc.gpsimd.collective_compute(
                    "AllGather",
                    mybir.AluOpType.bypass,
                    replica_groups=[list(range(NCORES))],
                    ins=[r_shard[:]],
                    outs=[r_all[:]],
                )
                for c in range(nch):
                    nc.sync.dma_start(
                        out=right_chunks[c],
                        in_=bass.AP(
                            tensor=r_all,
                            offset=c * tpc * P,
                            ap=[[1, P], [P, tpc]],
                        ),
                    )
                    emit_rel_dma(c)
            elif dots == "pe":
                first_mm = [True]
                # right dots in chunks, each evacuated to SBUF as soon as
                # ready so the main loop starts after chunk 0 (not all 64)
                for ch_i in range(nch):
                    for tt in range(tpc):
                        jt = ch_i * tpc + tt
                        if tt == 0:
                            # ctxT DRAM layout is [p, chunk, kt, j']: one
                            # contiguous nk*tpc*P run per partition
                            cht = ctxT_pool.tile(
                                [P, nk, tpc * P], bf16, tag="ctxT"
                            )
                            nc.sync.dma_start(
                                out=cht,
                                in_=bass.AP(
                                    tensor=ctxT,
                                    offset=ch_i * nk * tpc * P,
                                    ap=[
                                        [nch * nk * tpc * P, P],
                                        [1, nk * tpc * P],
                                    ],
                                ),
                            )
                        for k in range(nk):
                            nc.tensor.matmul(
                                shared[:, RD0 + jt : RD0 + jt + 1],
                                lhsT=cht[:, k, tt * P : (tt + 1) * P],
                                rhs=vrb[:, k : k + 1],
                                start=first_mm[0],  # zeroes bank 7 once/rep
                                stop=False,
                                skip_group_check=True,
                            )
                            first_mm[0] = False
                    nc.vector.tensor_copy(
                        right_chunks[ch_i],
                        shared[:, RD0 + ch_i * tpc : RD0 + (ch_i + 1) * tpc],
                    )
                    emit_rel_dma(ch_i)
            else:
                # DVE STT dots from packed ctx_dve; acc bank 7 is then a
                # normal accumulator (no shared-bank trickery at all)
                vrb_f = singles.tile([P, IN], bf16, tag="vrb_f")
                nc.sync.dma_start(
                    out=vrb_f,
                    in_=bass.AP(tensor=vr_in, offset=0, ap=[[0, P], [1, IN]]),
                )
                for ch_i in range(nch):
                    cdt = ctxT_pool.tile([P, tpc, IN], bf16, tag="ctxT")
                    nc.sync.dma_start(
                        out=cdt,
                        in_=bass.AP(
                            tensor=ctx_dve,
                            offset=ch_i * tpc * IN,
                            ap=[[njt * IN, P], [1, tpc * IN]],
                        ),
                    )
                    for tt in range(tpc):
                        jt = ch_i * tpc + tt
                        scr = scr_pool.tile([P, IN], f32, tag="scr")
                        nc.vector.scalar_tensor_tensor(
                            out=scr, in0=cdt[:, tt, :], scalar=0.0, in1=vrb_f,
                            op0=mybir.AluOpType.bypass,
                            op1=mybir.AluOpType.mult,
                            accum_out=right_col(jt),
                        )
                    emit_rel_dma(ch_i)

            # left + (b_l + b_r) -> fp16, bounce via DRAM, broadcast back
            lc16 = singles.tile([P, ni], f16, tag="lc16")
            nc.vector.tensor_scalar(
                out=lc16, in0=left_cols, scalar1=lbb[:, 0:1], scalar2=None,
                op0=mybir.AluOpType.add,
            )
            # bounce + broadcast ride the Pool SWDGE queue: their sem waits
            # must not block the ACT sequencer (exps) or SP queue (bulk DMA)
            bq = cfg.get("bounce_queue", "act")
    
            bounce_eng = {"pool": nc.gpsimd, "act": nc.scalar, "sp": nc.sync}[bq]
            bounce_eng.dma_start(
                out=bass.AP(tensor=l_scr, offset=0, ap=[[1, P], [P, ni]]),
                in_=lc16,
            )
            left_b = singles.tile([P, g, r], f16, tag="left_b")
            for t in range(g):
                bounce_eng.dma_start(
                    out=left_b[:, t, :],
                    in_=bass.AP(tensor=l_scr, offset=0, ap=[[0, P], [1, r]]),
                )

            # ------------------------- main loop ----------------------------
            # which groups use the 1-exp variant / pool u-add (spread evenly)
            n_1exp_done = 0
            n_pool_done = 0
            pending = []  # software pipeline: consume stage delayed 1 group

            def consume(ent):
                # 2-exp groups: the DVE max is emitted here, one iteration
                # after its exps, so DVE's in-order queue never stalls on ACT
                j0, gw, kind, tiles = ent
                if kind == "2exp":
                    e1t, e2t, zmt = tiles
                    nc.vector.tensor_max(zmt, e1t, e2t)
                else:
                    zmt = tiles[0]
                for tt in range(gw):
                    jt = j0 + tt
                    for ib in range(ni):
                        nc.tensor.matmul(
                            accs[ib][:, 0 : D + 1],
                            lhsT=zmt[:, tt, ib * P : (ib + 1) * P],
                            rhs=rel_tile(jt),
                            start=(jt == 0 and (ib < ni - 1 or dots != "pe")),
                            stop=(jt == njt - 1),
                            skip_group_check=(ib == ni - 1 and dots == "pe"),
                        )

            for gi in range(ngr):
                if gi + adj_pref < ngr:
                    emit_adjm_dma(gi + adj_pref)
                adjt, gw = adj_tiles.pop(gi)
                j0 = gspecs[gi][0]
                # u = (adjm + right_j) + left_i fused in one STT pass
                ut = u_pool.tile([P, gw, r], f16, tag="u")
                for tt in range(gw):
                    jt = j0 + tt
                    nc.vector.scalar_tensor_tensor(
                        out=ut[:, tt, :], in0=adjt[:, tt, :],
                        scalar=right_col(jt), in1=left_b[:, tt, :],
                        op0=mybir.AluOpType.add, op1=mybir.AluOpType.add,
                    )

                use_1exp = (n1exp * (gi + 1)) // ngr > n_1exp_done
                zmt = zm_pool.tile([P, gw, r], f16, tag="zm")
                if use_1exp:
                    # DVE-heavy: leaky via TS(0.2u) + max, single ACT exp
                    n_1exp_done += 1
                    at = e1_pool.tile([P, gw, r], f16, tag="e1")
                    nc.vector.tensor_scalar(
                        out=at, in0=ut, scalar1=0.2, scalar2=None,
                        op0=mybir.AluOpType.mult,
                    )
                    tt_ = e2_pool.tile([P, gw, r], f16, tag="e2")
                    nc.vector.tensor_max(tt_, ut, at)
                    nc.scalar.activation(
                        zmt, tt_, mybir.ActivationFunctionType.Exp,
                        bias=0.0, scale=1.0,
                    )
                    pending.append((j0, gw, "1exp", (zmt,)))
                elif use_prelu:
                    # ACT-only: parametric_relu (alpha=0.2) + exp, both in
                    # the exp table set -> no table reload, no DVE max
                    t16 = e1_pool.tile([P, gw, r], f16, tag="e1")
                    nc.scalar.activation(
                        t16, ut, mybir.ActivationFunctionType.Prelu,
                        bias=0.0, scale=1.0, alpha=0.2,
                    )
                    nc.scalar.activation(
                        zmt, t16, mybir.ActivationFunctionType.Exp,
                        bias=0.0, scale=1.0,
                    )
                    pending.append((j0, gw, "1exp", (zmt,)))
                else:
                    e1t = e1_pool.tile([P, gw, r], f16, tag="e1")
                    nc.scalar.activation(
                        e1t, ut, mybir.ActivationFunctionType.Exp,
                        bias=0.0, scale=1.0,
                    )
                    e2t = e2_pool.tile([P, gw, r], f16, tag="e2")
                    nc.scalar.activation(
                        e2t, ut, mybir.ActivationFunctionType.Exp,
                        bias=0.0, scale=0.2,
                    )
                    pending.append((j0, gw, "2exp", (e1t, e2t, zmt)))

                if len(pending) > 1:
                    consume(pending.pop(0))
            while pending:
                consume(pending.pop(0))

            # ------------------------ finalize ------------------------------
            # finalize entirely on DVE: relu(num/den) = max(num*recip, 0)
            # (recip > 0), avoiding ACT table swaps and engine ping-pong
            for ib in range(ni):
                recip = sm_pool.tile([P, 1], f32, tag="recip")
                nc.vector.reciprocal(recip, accs[ib][:, D : D + 1])
                ob = out_pool.tile([P, D], f16, tag="ob")
                nc.vector.tensor_scalar(
                    out=ob, in0=accs[ib][:, 0:D], scalar1=recip[:, 0:1],
                    scalar2=0.0, op0=mybir.AluOpType.mult,
                    op1=mybir.AluOpType.max,
                )
                nc.sync.dma_start(out=out[ib * P : (ib + 1) * P, :], in_=ob)

        flat_reps = cfg.get("flat_reps", 1)  # sim-only: unrolled reps
        if reps > 1:
            with tc.For_i(0, reps, 1):
                _emit_body()
        else:
            for _ in range(flat_reps):
                _emit_body()

    nc.compile()
    return nc


_BASE_CFG = dict(n=N, r=N // NCORES, g=4, n1exp=8, pool_add=0, dots="pe",
                 pref=2, use_prelu=True, dot_chunks=8, head_split=[2, 2],
                 tail_split=[2, 1, 1], e_bufs=4)


def _get_program(cfg_key):
    if cfg_key not in _CACHE:
        _CACHE[cfg_key] = build_program(dict(_BASE_CFG))
    return _CACHE[cfg_key]


def prepare_in_maps(relation, context, adj_tensor, W_common, w_left, b_left,
                    w_right, b_right):
    relation = np.asarray(relation, dtype=np.float32)
    context = np.asarray(context, dtype=np.float32)
    adj_tensor = np.asarray(adj_tensor, dtype=np.float32)
    W_common = np.asarray(W_common, dtype=np.float32)
    w_left = np.asarray(w_left, dtype=np.float32)
    w_right = np.asarray(w_right, dtype=np.float32)
    b_l = float(np.asarray(b_left))
    b_r = float(np.asarray(b_right))

    # host-side parameter folding (weights only, no activations)
    v_left = (W_common.T @ w_left).astype(ml_dtypes.bfloat16)
    v_right = (W_common.T @ w_right).astype(ml_dtypes.bfloat16)
    lbias = np.array([b_l + b_r], dtype=np.float32)

    relh = relation.astype(np.float16)
    ctxb = context.astype(ml_dtypes.bfloat16)
    # ctxT packed as [p, chunk, kt, j']: partition-contiguous chunk reads
    nch = _BASE_CFG.get("dot_chunks", 8)
    tpcP = N // nch  # j per chunk
    ctxT8 = np.ascontiguousarray(
        ctxb.T.reshape(4, P, nch, tpcP).transpose(1, 2, 0, 3)
    )  # [128, nch, 4, tpcP]

    # adjm[j, i] = MASKB * (adj[i, j] - 1):  0 if edge, -512 if masked
    adjm_full = ((adj_tensor - 1.0) * MASKB).astype(ml_dtypes.float8_e5m2)

    rows = N // NCORES
    njt = N // P
    in_maps = []
    for c in range(NCORES):
        sl = slice(c * rows, (c + 1) * rows)
        # adjm slab [j, i] -> [p, jt, i] (partition-contiguous group reads)
        aT = adjm_full[sl].T.reshape(njt, P, rows)
        m = {
            "adjm": np.ascontiguousarray(aT.transpose(1, 0, 2)),
            "ctx_own": np.ascontiguousarray(
                ctxb[sl].reshape(rows // P, P, IN).transpose(1, 0, 2)
            ),
            "ctxT": ctxT8,
            "rel_in": relh,
            "vl_in": v_left,
            "vr_in": v_right,
            "lbias": lbias,
        }
        in_maps.append(m)
    return in_maps


# ------------------------------------------------------------------- entry
def kernel(relation, context, adj_tensor, W_common, w_left, b_left, w_right,
           b_right):
    from concourse.bass_utils import run_bass_kernel_spmd

    in_maps = prepare_in_maps(relation, context, adj_tensor, W_common,
                              w_left, b_left, w_right, b_right)
    nc = _get_program("main")
    last_err = None
    for _attempt in range(3):
        try:
            res = run_bass_kernel_spmd(nc, in_maps, list(range(NCORES)))
            outs = [np.asarray(res.results[c]["out"]) for c in range(NCORES)]
            return np.concatenate(outs, axis=0).astype(np.float32)
        except Exception as e:  # transient device-unrecoverable seen on axon
            last_err = e
            import time as _time

            try:
                import jax

                jax.clear_caches()
            except Exception:
                pass
            _time.sleep(3.0)
    raise last_err


# revision 37
# speedup vs baseline: 1.4435x; 1.0350x over previous
"""Trainium2 Bass kernel for nn_DenseAttentionLayer (gnn_message_passing).

Math (reference):
    in_fts = context @ W_common.T            # (N, HID)
    left   = in_fts @ w_left + b_left        # (N,)
    right  = in_fts @ w_right + b_right      # (N,)
    logits = leaky_relu(left[:,None] + right[None,:], 0.2)
    logits = where(adj <= 0, -inf, logits)
    coefs  = softmax(logits, axis=-1)
    out    = relu(coefs @ relation)          # (N, REL_DIM)

Design (v5: transposed layout, fp8 mask, Prelu):
  * left = context @ (W_common.T @ w_left) + b_left (host-folded weights).
  * Layout: partition dim = j (columns of the NxN logits), free dim = i
    (the core's own rows).  adj is host-transformed per core into
    adjm[j, i] = 512*(adj-1) in fp8_e5m2 ({0, -512} exactly), packed
    [p, jt, i] so each partition's group slab is one contiguous DMA run.
    Then u = left_i + right_j + adjm is the exact logits for unmasked
    entries and <= -500 for masked ones, so exp underflows masked
    entries to exactly 0 - no separate mask multiply, no row-max pass
    (logits are O(1)), and the mask stream is 1 byte/element.  This
    kernel is strongly DMA-bound on the axon TRN2 target, so mask bytes
    dominate the runtime.
  * u is built in ONE scalar_tensor_tensor pass (adjm + right_j scalar +
    left_i broadcast).  STT always runs 1x on DVE (no 2-byte fast mode),
    which the fp8 operand would forfeit anyway - so the fuse is free.
  * exp(leaky(x)): two per-group variants balance ACT vs DVE load:
      - prelu: t = Prelu(u, alpha=0.2); zm = Exp(t).  parametric_relu
        honors alpha on this HW and shares the exp activation table
        (no table reloads).  All on ACT.
      - 1exp: a = 0.2u (TS 4x), t = max(u,a) (TT 2x) on DVE; Exp on ACT.
  * zm tiles [j, i] in fp16 are directly the matmul lhsT:
    acc[i,:] += zm.T @ rel_aug[j,:] - no PE transposes, no PSUM
    evacuations.  Softmax denominator comes free as column 256 of the
    matmul (ones column on rel_aug).  8 PSUM banks = 8 accumulators.
  * left/right dot products (ctx @ v) run on the PE from host-transposed
    ctxT tiles into the spare region of PSUM bank 7 (shared with acc7:
    matmul start=True zeroes the whole 2KB bank, so exactly one dots
    matmul per rep starts; everything else accumulates with
    skip_group_check).  Dots are evacuated to per-chunk SBUF tiles
    (dep tracking is tile-granular) and the mask STT reads right_j from
    SBUF as its per-partition scalar.
  * DMA queue split: adjm + left-bounce ride the ACT HWDGE queue,
    ctxT/rel/ctx_own the SP queue (a slow Pool-SWDGE experiment lost).
  * Emission is software-pipelined (consume stage delayed one group) so
    DVE's in-order queue never stalls on ACT output; group widths taper
    at head/tail ([2,2]...[2,1,1]) for faster ramp and drain.
  * finalize entirely on DVE: relu(num/den) = max(num*recip, 0), written
    out as fp16 (upconverted on host).

Sharding (8 cores): row-shard the N x N logits; each core owns r = N/8
rows (i), sees all N columns (j).  All params + rel + ctx replicated.
Measured on axon TRN2: ~165 us/iter (baseline: 312-484 us), rel err
2.9e-3 (tolerance 2e-2).
"""

import os
import sys

for _p in ("/opt/trn_rl_repo",):
    if _p not in sys.path and os.path.isdir(_p):
        sys.path.insert(0, _p)

from contextlib import ExitStack

import ml_dtypes
import numpy as np

# ---------------------------------------------------------------- constants
N = 8192  # num relations
IN = 512  # 2 * entity dim (context feature dim)
D = 256  # relation dim (output dim)
NCORES = 8
P = 128
MASKB = 512.0  # mask offset: adjm = MASKB*(adj-1), masked -> -512

_CACHE = {}


# ------------------------------------------------------------------ builder
def build_program(cfg):
    """Build the SPMD single-core Bass program."""
    import concourse.bass as bass
    import concourse.tile as tile
    from concourse import bacc, mybir

    f32 = mybir.dt.float32
    bf16 = mybir.dt.bfloat16
    f16 = mybir.dt.float16

    n = cfg["n"]  # full N (j extent)
    r = cfg["r"]  # rows per core (i extent)
    g = cfg["g"]  # j-tiles per group
    reps = cfg.get("reps", 1)  # >1: loop whole kernel (timing harness only)
    n1exp = cfg.get("n1exp", 2)  # of ng groups, how many use 1-exp variant
    pool_add = cfg.get("pool_add", 10)  # of ng groups: u-add on gpsimd
    dots = cfg.get("dots", "pe")  # 'pe' | 'dve'
    pref = cfg.get("pref", 2)  # dots group lookahead
    use_prelu = cfg.get("use_prelu", True)  # parametric_relu honors alpha

    ni = r // P  # i-blocks per core (8)
    njt = n // P  # j-tiles (64)
    ng = njt // g  # groups
    nk = IN // P  # k-tiles (4)

    assert ni == 8
    # shared PSUM bank 7 layout (f32 cols): acc7 [0:257], right dots
    # [257:257+njt], left dots [257+njt : 257+njt+ni]
    RD0 = D + 1
    LD0 = RD0 + njt
    assert LD0 + ni <= 512

    nc = bacc.Bacc("TRN2", target_bir_lowering=False, debug=False)

    f8 = mybir.dt.float8e5
    adjm = nc.dram_tensor("adjm", [n, r], f8, kind="ExternalInput")
    ctx_own = nc.dram_tensor("ctx_own", [r, IN], bf16, kind="ExternalInput")
    if cfg.get("dots", "pe") == "dve":
        ctx_dve = nc.dram_tensor("ctx_dve", [n, IN], bf16, kind="ExternalInput")
    ctxT = nc.dram_tensor("ctxT", [IN, n], bf16, kind="ExternalInput")
    rel_in = nc.dram_tensor("rel_in", [n, D + 1], f16, kind="ExternalInput")
    vl_in = nc.dram_tensor("vl_in", [IN], bf16, kind="ExternalInput")
    vr_in = nc.dram_tensor("vr_in", [IN], bf16, kind="ExternalInput")
    lbias = nc.dram_tensor("lbias", [1], f32, kind="ExternalInput")  # b_l+b_r
    out = nc.dram_tensor("out", [r, D], f16, kind="ExternalOutput")
    l_scr = nc.dram_tensor("l_scr", [r], f16)  # left bounce scratch
    use_ag = cfg.get("use_ag", False)
    if use_ag:
        r_shard = nc.dram_tensor("r_shard", [r], f32)
        r_all = nc.dram_tensor("r_all", [n], f32, addr_space="Shared")

    with tile.TileContext(nc) as tc, ExitStack() as ctx:
        singles = ctx.enter_context(tc.tile_pool(name="singles", bufs=1))
        ctxT_pool = ctx.enter_context(tc.tile_pool(name="ctxTp", bufs=2))
        adj_pool = ctx.enter_context(tc.tile_pool(name="adjp", bufs=cfg.get("adj_bufs", 3)))
        u_pool = ctx.enter_context(tc.tile_pool(name="up", bufs=3))
        e1_pool = ctx.enter_context(tc.tile_pool(name="e1p", bufs=cfg.get("e_bufs", 4)))
        e2_pool = ctx.enter_context(tc.tile_pool(name="e2p", bufs=cfg.get("e_bufs", 4)))
        zm_pool = ctx.enter_context(tc.tile_pool(name="zmp", bufs=3))
        out_pool = ctx.enter_context(tc.tile_pool(name="outp", bufs=2))
        sm_pool = ctx.enter_context(tc.tile_pool(name="smp", bufs=2))
        acc_psum = ctx.enter_context(
            tc.tile_pool(name="accps", bufs=1, space="PSUM")
        )
        scr_pool = ctx.enter_context(tc.tile_pool(name="scrp", bufs=1))

        def _emit_body():
            # group spec: (start j-tile, width); smaller groups at the head
            # (faster ramp) and tail (shorter pipeline drain)
            gspecs = []
            pos = 0
            head = cfg.get("head_split", [2, 2])
            tail = cfg.get("tail_split", [2, 1, 1])
            for w in head:
                gspecs.append((pos, w)); pos += w
            while pos < njt - sum(tail):
                gspecs.append((pos, g)); pos += g
            for w in tail:
                gspecs.append((pos, w)); pos += w
            assert pos == njt
            ngr = len(gspecs)

            adj_tiles = {}

            adj_eng = nc.scalar if cfg.get("adj_queue", "act") == "act" else nc.sync

            def emit_adjm_dma(gi):
                # adjm DRAM layout is [p, jt, i] so each partition's group
                # slab is one contiguous run (1 DMA descriptor/partition)
                j0, gw = gspecs[gi]
                adjt = adj_pool.tile([P, gw, r], f8, tag="adj")
                adj_eng.dma_start(
                    out=adjt,
                    in_=bass.AP(
                        tensor=adjm,
                        offset=j0 * r,
                        ap=[[njt * r, P], [r, gw], [1, r]],
                    ),
                )
                adj_tiles[gi] = (adjt, gw)

            adj_pref = cfg.get("adj_pref", 2)
            for _gi in range(min(adj_pref, ngr)):
                emit_adjm_dma(_gi)

            # ---------------- phase 0: params ----------------
            vlb = singles.tile([P, nk], bf16, tag="vlb")
            nc.sync.dma_start(
                out=vlb, in_=bass.AP(tensor=vl_in, offset=0, ap=[[1, P], [P, nk]])
            )
            vrb = singles.tile([P, nk], bf16, tag="vrb")
            nc.sync.dma_start(
                out=vrb, in_=bass.AP(tensor=vr_in, offset=0, ap=[[1, P], [P, nk]])
            )
            lbb = singles.tile([P, 1], f32, tag="lbb")
            nc.sync.dma_start(
                out=lbb, in_=bass.AP(tensor=lbias, offset=0, ap=[[0, P], [1, 1]])
            )

            # relation chunks, each augmented with a ones column
            # (denominator trick).  Per-chunk tiles + deferred DMA keep the
            # big rel read off the critical path and the deps fine-grained.
            nch = cfg.get("dot_chunks", 8)
            tpc = njt // nch  # j-tiles per chunk
            rel_chunks = [
                singles.tile([P, tpc, D + 1], f16, name=f"relch{c}", tag=f"relch{c}")
                for c in range(nch)
            ]

            rel_eng = {"sp": nc.sync, "act": nc.scalar, "pool": nc.gpsimd}[
                cfg.get("rel_queue", "act")
            ]

            def emit_rel_dma(c):
                # rel ships host-packed [p, jt, D+1] with the ones column
                # baked in: one contiguous descriptor per partition
                rel_eng.dma_start(
                    out=rel_chunks[c],
                    in_=bass.AP(
                        tensor=rel_in,
                        offset=c * tpc * (D + 1),
                        ap=[[njt * (D + 1), P], [1, tpc * (D + 1)]],
                    ),
                )

            def rel_tile(jt):
                return rel_chunks[jt // tpc][:, jt % tpc, :]

            # PSUM accumulators: banks 0-6 own i-blocks 0-6; bank 7 shared
            # between acc7 and the left/right dot-product columns.
            accs = [
                acc_psum.tile([P, 512], f32, tag=f"acc{ib}", name=f"acc{ib}")
                for ib in range(ni - 1)
            ]
            shared = acc_psum.tile([P, 512], f32, tag="accsh", name="accsh")
            accs.append(shared)

            # ---------------- dot products (prologue) ----------------
            # left/right dots accumulate in the spare region of shared PSUM
            # bank 7 and are evacuated to SBUF in chunks.  Each chunk gets
            # its OWN SBUF tile: dependency tracking is tile-granular, so a
            # single shared tile would make the first main-loop read wait
            # for the LAST chunk's copy.
            left_sb = singles.tile([P, ni], f32, tag="left_sb")
            left_cols = left_sb[:, :]
            right_chunks = [
                singles.tile([P, tpc], f32, name=f"rchunk{c}", tag=f"rchunk{c}")
                for c in range(nch)
            ]

            def right_col(jt):
                return right_chunks[jt // tpc][:, jt % tpc : jt % tpc + 1]


            # left dots on DVE (STT accum) from natural-layout ctx_own:
            # DVE is idle at program start and this keeps the left_b chain
            # (lc16 -> DRAM bounce -> broadcast), which gates the first
            # u-add, off the PE/prologue critical path.
            vlb_f = singles.tile([P, IN], bf16, tag="vlb_f")
            nc.sync.dma_start(
                out=vlb_f,
                in_=bass.AP(tensor=vl_in, offset=0, ap=[[0, P], [1, IN]]),
            )
            cow = singles.tile([P, ni, IN], bf16, tag="cow")
            nc.sync.dma_start(
                out=cow,
                in_=bass.AP(
                    tensor=ctx_own, offset=0, ap=[[ni * IN, P], [1, ni * IN]]
                ),
            )
            for t in range(ni):
                scr = scr_pool.tile([P, IN], f32, tag="scr")
                nc.vector.scalar_tensor_tensor(
                    out=scr, in0=cow[:, t, :], scalar=0.0, in1=vlb_f,
                    op0=mybir.AluOpType.bypass, op1=mybir.AluOpType.mult,
                    accum_out=left_cols[:, t : t + 1],
                )

            if use_ag:
                # right dots for OWN rows only (from cow, like left), then
                # AllGather the 8192-float right vector (32KB) instead of
                # re-reading the full replicated ctxT (8.4MB per core)
                vrb_f = singles.tile([P, IN], bf16, tag="vrb_f")
                nc.sync.dma_start(
                    out=vrb_f,
                    in_=bass.AP(tensor=vr_in, offset=0, ap=[[0, P], [1, IN]]),
                )
                r_own = singles.tile([P, ni], f32, tag="r_own")
                for t in range(ni):
                    scr = scr_pool.tile([P, IN], f32, tag="scr")
                    nc.vector.scalar_tensor_tensor(
                        out=scr, in0=cow[:, t, :], scalar=0.0, in1=vrb_f,
                        op0=mybir.AluOpType.bypass, op1=mybir.AluOpType.mult,
                        accum_out=r_own[:, t : t + 1],
                    )
                nc.sync.dma_start(
                    out=bass.AP(tensor=r_shard, offset=0, ap=[[1, P], [P, ni]]),
                    in_=r_own,
                )
                nc.gpsimd.collective_compute(
                    "AllGather",
                    mybir.AluOpType.bypass,
                    replica_groups=[list(range(NCORES))],
                    ins=[r_shard[:]],
                    outs=[r_all[:]],
                )
                for c in range(nch):
                    nc.sync.dma_start(
                        out=right_chunks[c],
                        in_=bass.AP(
                            tensor=r_all,
                            offset=c * tpc * P,
                            ap=[[1, P], [P, tpc]],
                        ),
                    )
                    emit_rel_dma(c)
            elif dots == "pe":
                first_mm = [True]
                # right dots in chunks, each evacuated to SBUF as soon as
                # ready so the main loop starts after chunk 0 (not all 64)
                for ch_i in range(nch):
                    for tt in range(tpc):
                        jt = ch_i * tpc + tt
                        if tt == 0:
                            # ctxT DRAM layout is [p, chunk, kt, j']: one
                            # contiguous nk*tpc*P run per partition
                            cht = ctxT_pool.tile(
                                [P, nk, tpc * P], bf16, tag="ctxT"
                            )
                            nc.sync.dma_start(
                                out=cht,
                                in_=bass.AP(
                                    tensor=ctxT,
                                    offset=ch_i * nk * tpc * P,
                                    ap=[
                                        [nch * nk * tpc * P, P],
                                        [1, nk * tpc * P],
                                    ],
                                ),
                            )
                        for k in range(nk):
                            nc.tensor.matmul(
                                shared[:, RD0 + jt : RD0 + jt + 1],
                                lhsT=cht[:, k, tt * P : (tt + 1) * P],
                                rhs=vrb[:, k : k + 1],
                                start=first_mm[0],  # zeroes bank 7 once/rep
                                stop=False,
                                skip_group_check=True,
                            )
                            first_mm[0] = False
                    nc.vector.tensor_copy(
                        right_chunks[ch_i],
                        shared[:, RD0 + ch_i * tpc : RD0 + (ch_i + 1) * tpc],
                    )
                    emit_rel_dma(ch_i)
            else:
                # DVE STT dots from packed ctx_dve; acc bank 7 is then a
                # normal accumulator (no shared-bank trickery at all)
                vrb_f = singles.tile([P, IN], bf16, tag="vrb_f")
                nc.sync.dma_start(
                    out=vrb_f,
                    in_=bass.AP(tensor=vr_in, offset=0, ap=[[0, P], [1, IN]]),
                )
                for ch_i in range(nch):
                    cdt = ctxT_pool.tile([P, tpc, IN], bf16, tag="ctxT")
                    nc.sync.dma_start(
                        out=cdt,
                        in_=bass.AP(
                            tensor=ctx_dve,
                            offset=ch_i * tpc * IN,
                            ap=[[njt * IN, P], [1, tpc * IN]],
                        ),
                    )
                    for tt in range(tpc):
                        jt = ch_i * tpc + tt
                        scr = scr_pool.tile([P, IN], f32, tag="scr")
                        nc.vector.scalar_tensor_tensor(
                            out=scr, in0=cdt[:, tt, :], scalar=0.0, in1=vrb_f,
                            op0=mybir.AluOpType.bypass,
                            op1=mybir.AluOpType.mult,
                            accum_out=right_col(jt),
                        )
                    emit_rel_dma(ch_i)

            # left + (b_l + b_r) -> fp16, bounce via DRAM, broadcast back
            lc16 = singles.tile([P, ni], f16, tag="lc16")
            nc.vector.tensor_scalar(
                out=lc16, in0=left_cols, scalar1=lbb[:, 0:1], scalar2=None,
                op0=mybir.AluOpType.add,
            )
            # bounce + broadcast ride the Pool SWDGE queue: their sem waits
            # must not block the ACT sequencer (exps) or SP queue (bulk DMA)
            bq = cfg.get("bounce_queue", "act")
    
            bounce_eng = {"pool": nc.gpsimd, "act": nc.scalar, "sp": nc.sync}[bq]
            bounce_eng.dma_start(
                out=bass.AP(tensor=l_scr, offset=0, ap=[[1, P], [P, ni]]),
                in_=lc16,
            )
            left_b = singles.tile([P, g, r], f16, tag="left_b")
            for t in range(g):
                bounce_eng.dma_start(
                    out=left_b[:, t, :],
                    in_=bass.AP(tensor=l_scr, offset=0, ap=[[0, P], [1, r]]),
                )

            # ------------------------- main loop ----------------------------
            # which groups use the 1-exp variant / pool u-add (spread evenly)
            n_1exp_done = 0
            n_pool_done = 0
            pending = []  # software pipeline: consume stage delayed 1 group

            def consume(ent):
                # 2-exp groups: the DVE max is emitted here, one iteration
                # after its exps, so DVE's in-order queue never stalls on ACT
                j0, gw, kind, tiles = ent
                if kind == "2exp":
                    e1t, e2t, zmt = tiles
                    nc.vector.tensor_max(zmt, e1t, e2t)
                else:
                    zmt = tiles[0]
                for tt in range(gw):
                    jt = j0 + tt
                    for ib in range(ni):
                        nc.tensor.matmul(
                            accs[ib][:, 0 : D + 1],
                            lhsT=zmt[:, tt, ib * P : (ib + 1) * P],
                            rhs=rel_tile(jt),
                            start=(jt == 0 and (ib < ni - 1 or dots != "pe")),
                            stop=(jt == njt - 1),
                            skip_group_check=(ib == ni - 1 and dots == "pe"),
                        )

            for gi in range(ngr):
                if gi + adj_pref < ngr:
                    emit_adjm_dma(gi + adj_pref)
                adjt, gw = adj_tiles.pop(gi)
                j0 = gspecs[gi][0]
                # u = (adjm + right_j) + left_i fused in one STT pass
                ut = u_pool.tile([P, gw, r], f16, tag="u")
                for tt in range(gw):
                    jt = j0 + tt
                    nc.vector.scalar_tensor_tensor(
                        out=ut[:, tt, :], in0=adjt[:, tt, :],
                        scalar=right_col(jt), in1=left_b[:, tt, :],
                        op0=mybir.AluOpType.add, op1=mybir.AluOpType.add,
                    )

                use_1exp = (n1exp * (gi + 1)) // ngr > n_1exp_done
                zmt = zm_pool.tile([P, gw, r], f16, tag="zm")
                if use_1exp:
                    # DVE-heavy: leaky via TS(0.2u) + max, single ACT exp
                    n_1exp_done += 1
                    at = e1_pool.tile([P, gw, r], f16, tag="e1")
                    nc.vector.tensor_scalar(
                        out=at, in0=ut, scalar1=0.2, scalar2=None,
                        op0=mybir.AluOpType.mult,
                    )
                    tt_ = e2_pool.tile([P, gw, r], f16, tag="e2")
                    nc.vector.tensor_max(tt_, ut, at)
                    nc.scalar.activation(
                        zmt, tt_, mybir.ActivationFunctionType.Exp,
                        bias=0.0, scale=1.0,
                    )
                    pending.append((j0, gw, "1exp", (zmt,)))
                elif use_prelu:
                    # ACT-only: parametric_relu (alpha=0.2) + exp, both in
                    # the exp table set -> no table reload, no DVE max
                    t16 = e1_pool.tile([P, gw, r], f16, tag="e1")
                    nc.scalar.activation(
                        t16, ut, mybir.ActivationFunctionType.Prelu,
                        bias=0.0, scale=1.0, alpha=0.2,
                    )
                    nc.scalar.activation(
                        zmt, t16, mybir.ActivationFunctionType.Exp,
                        bias=0.0, scale=1.0,
                    )
                    pending.append((j0, gw, "1exp", (zmt,)))
                else:
                    e1t = e1_pool.tile([P, gw, r], f16, tag="e1")
                    nc.scalar.activation(
                        e1t, ut, mybir.ActivationFunctionType.Exp,
                        bias=0.0, scale=1.0,
                    )
                    e2t = e2_pool.tile([P, gw, r], f16, tag="e2")
                    nc.scalar.activation(
                        e2t, ut, mybir.ActivationFunctionType.Exp,
                        bias=0.0, scale=0.2,
                    )
                    pending.append((j0, gw, "2exp", (e1t, e2t, zmt)))

                if len(pending) > 1:
                    consume(pending.pop(0))
            while pending:
                consume(pending.pop(0))

            # ------------------------ finalize ------------------------------
            # finalize entirely on DVE: relu(num/den) = max(num*recip, 0)
            # (recip > 0), avoiding ACT table swaps and engine ping-pong
            for ib in range(ni):
                recip = sm_pool.tile([P, 1], f32, tag="recip")
                nc.vector.reciprocal(recip, accs[ib][:, D : D + 1])
                ob = out_pool.tile([P, D], f16, tag="ob")
                nc.vector.tensor_scalar(
                    out=ob, in0=accs[ib][:, 0:D], scalar1=recip[:, 0:1],
                    scalar2=0.0, op0=mybir.AluOpType.mult,
                    op1=mybir.AluOpType.max,
                )
                nc.sync.dma_start(out=out[ib * P : (ib + 1) * P, :], in_=ob)

        flat_reps = cfg.get("flat_reps", 1)  # sim-only: unrolled reps
        if reps > 1:
            with tc.For_i(0, reps, 1):
                _emit_body()
        else:
            for _ in range(flat_reps):
                _emit_body()

    nc.compile()
    return nc


_BASE_CFG = dict(n=N, r=N // NCORES, g=4, n1exp=8, pool_add=0, dots="pe",
                 pref=2, use_prelu=True, dot_chunks=8, head_split=[2, 2],
                 tail_split=[2, 1, 1], e_bufs=4)


def _get_program(cfg_key):
    if cfg_key not in _CACHE:
        _CACHE[cfg_key] = build_program(dict(_BASE_CFG))
    return _CACHE[cfg_key]


def prepare_in_maps(relation, context, adj_tensor, W_common, w_left, b_left,
                    w_right, b_right):
    relation = np.asarray(relation, dtype=np.float32)
    context = np.asarray(context, dtype=np.float32)
    adj_tensor = np.asarray(adj_tensor, dtype=np.float32)
    W_common = np.asarray(W_common, dtype=np.float32)
    w_left = np.asarray(w_left, dtype=np.float32)
    w_right = np.asarray(w_right, dtype=np.float32)
    b_l = float(np.asarray(b_left))
    b_r = float(np.asarray(b_right))

    # host-side parameter folding (weights only, no activations)
    v_left = (W_common.T @ w_left).astype(ml_dtypes.bfloat16)
    v_right = (W_common.T @ w_right).astype(ml_dtypes.bfloat16)
    lbias = np.array([b_l + b_r], dtype=np.float32)

    relh = np.concatenate(
        [relation, np.ones((N, 1), np.float32)], axis=1
    ).astype(np.float16)
    relh = np.ascontiguousarray(
        relh.reshape(N // P, P, D + 1).transpose(1, 0, 2)
    )  # [p, jt, D+1]
    ctxb = context.astype(ml_dtypes.bfloat16)
    # ctxT packed as [p, chunk, kt, j']: partition-contiguous chunk reads
    nch = _BASE_CFG.get("dot_chunks", 8)
    tpcP = N // nch  # j per chunk
    ctxT8 = np.ascontiguousarray(
        ctxb.T.reshape(4, P, nch, tpcP).transpose(1, 2, 0, 3)
    )  # [128, nch, 4, tpcP]

    # adjm[j, i] = MASKB * (adj[i, j] - 1):  0 if edge, -512 if masked
    adjm_full = ((adj_tensor - 1.0) * MASKB).astype(ml_dtypes.float8_e5m2)

    rows = N // NCORES
    njt = N // P
    in_maps = []
    for c in range(NCORES):
        sl = slice(c * rows, (c + 1) * rows)
        # adjm slab [j, i] -> [p, jt, i] (partition-contiguous group reads)
        aT = adjm_full[sl].T.reshape(njt, P, rows)
        m = {
            "adjm": np.ascontiguousarray(aT.transpose(1, 0, 2)),
            "ctx_own": np.ascontiguousarray(
                ctxb[sl].reshape(rows // P, P, IN).transpose(1, 0, 2)
            ),
            "ctxT": ctxT8,
            "rel_in": relh,
            "vl_in": v_left,
            "vr_in": v_right,
            "lbias": lbias,
        }
        in_maps.append(m)
    return in_maps


# ------------------------------------------------------------------- entry
def kernel(relation, context, adj_tensor, W_common, w_left, b_left, w_right,
           b_right):
    from concourse.bass_utils import run_bass_kernel_spmd

    in_maps = prepare_in_maps(relation, context, adj_tensor, W_common,
                              w_left, b_left, w_right, b_right)
    nc = _get_program("main")
    last_err = None
    for _attempt in range(3):
        try:
            res = run_bass_kernel_spmd(nc, in_maps, list(range(NCORES)))
            outs = [np.asarray(res.results[c]["out"]) for c in range(NCORES)]
            return np.concatenate(outs, axis=0).astype(np.float32)
        except Exception as e:  # transient device-unrecoverable seen on axon
            last_err = e
            import time as _time

            try:
                import jax

                jax.clear_caches()
            except Exception:
                pass
            _time.sleep(3.0)
    raise last_err


# revision 38
# speedup vs baseline: 1.4857x; 1.0292x over previous
"""Trainium2 Bass kernel for nn_DenseAttentionLayer (gnn_message_passing).

Math (reference):
    in_fts = context @ W_common.T            # (N, HID)
    left   = in_fts @ w_left + b_left        # (N,)
    right  = in_fts @ w_right + b_right      # (N,)
    logits = leaky_relu(left[:,None] + right[None,:], 0.2)
    logits = where(adj <= 0, -inf, logits)
    coefs  = softmax(logits, axis=-1)
    out    = relu(coefs @ relation)          # (N, REL_DIM)

Design (v5: transposed layout, fp8 mask, Prelu):
  * left = context @ (W_common.T @ w_left) + b_left (host-folded weights).
  * Layout: partition dim = j (columns of the NxN logits), free dim = i
    (the core's own rows).  adj is host-transformed per core into
    adjm[j, i] = 512*(adj-1) in fp8_e5m2 ({0, -512} exactly), packed
    [p, jt, i] so each partition's group slab is one contiguous DMA run.
    Then u = left_i + right_j + adjm is the exact logits for unmasked
    entries and <= -500 for masked ones, so exp underflows masked
    entries to exactly 0 - no separate mask multiply, no row-max pass
    (logits are O(1)), and the mask stream is 1 byte/element.  This
    kernel is strongly DMA-bound on the axon TRN2 target, so mask bytes
    dominate the runtime.
  * u is built in ONE scalar_tensor_tensor pass (adjm + right_j scalar +
    left_i broadcast).  STT always runs 1x on DVE (no 2-byte fast mode),
    which the fp8 operand would forfeit anyway - so the fuse is free.
  * exp(leaky(x)): two per-group variants balance ACT vs DVE load:
      - prelu: t = Prelu(u, alpha=0.2); zm = Exp(t).  parametric_relu
        honors alpha on this HW and shares the exp activation table
        (no table reloads).  All on ACT.
      - 1exp: a = 0.2u (TS 4x), t = max(u,a) (TT 2x) on DVE; Exp on ACT.
  * zm tiles [j, i] in fp16 are directly the matmul lhsT:
    acc[i,:] += zm.T @ rel_aug[j,:] - no PE transposes, no PSUM
    evacuations.  Softmax denominator comes free as column 256 of the
    matmul (ones column on rel_aug).  8 PSUM banks = 8 accumulators.
  * left/right dot products (ctx @ v) run on the PE from host-transposed
    ctxT tiles into the spare region of PSUM bank 7 (shared with acc7:
    matmul start=True zeroes the whole 2KB bank, so exactly one dots
    matmul per rep starts; everything else accumulates with
    skip_group_check).  Dots are evacuated to per-chunk SBUF tiles
    (dep tracking is tile-granular) and the mask STT reads right_j from
    SBUF as its per-partition scalar.
  * DMA queue split: adjm + left-bounce ride the ACT HWDGE queue,
    ctxT/rel/ctx_own the SP queue (a slow Pool-SWDGE experiment lost).
  * Emission is software-pipelined (consume stage delayed one group) so
    DVE's in-order queue never stalls on ACT output; group widths taper
    at head/tail ([2,2]...[2,1,1]) for faster ramp and drain.
  * finalize entirely on DVE: relu(num/den) = max(num*recip, 0), written
    out as fp16 (upconverted on host).

Sharding (8 cores): row-shard the N x N logits; each core owns r = N/8
rows (i), sees all N columns (j).  All params + rel + ctx replicated.
Measured on axon TRN2: ~165 us/iter (baseline: 312-484 us), rel err
2.9e-3 (tolerance 2e-2).
"""

import os
import sys

for _p in ("/opt/trn_rl_repo",):
    if _p not in sys.path and os.path.isdir(_p):
        sys.path.insert(0, _p)

from contextlib import ExitStack

import ml_dtypes
import numpy as np

# ---------------------------------------------------------------- constants
N = 8192  # num relations
IN = 512  # 2 * entity dim (context feature dim)
D = 256  # relation dim (output dim)
NCORES = 8
P = 128
MASKB = 512.0  # mask offset: adjm = MASKB*(adj-1), masked -> -512

_CACHE = {}


# ------------------------------------------------------------------ builder
def build_program(cfg):
    """Build the SPMD single-core Bass program."""
    import concourse.bass as bass
    import concourse.tile as tile
    from concourse import bacc, mybir

    f32 = mybir.dt.float32
    bf16 = mybir.dt.bfloat16
    f16 = mybir.dt.float16

    n = cfg["n"]  # full N (j extent)
    r = cfg["r"]  # rows per core (i extent)
    g = cfg["g"]  # j-tiles per group
    reps = cfg.get("reps", 1)  # >1: loop whole kernel (timing harness only)
    n1exp = cfg.get("n1exp", 2)  # of ng groups, how many use 1-exp variant
    pool_add = cfg.get("pool_add", 10)  # of ng groups: u-add on gpsimd
    dots = cfg.get("dots", "pe")  # 'pe' | 'dve'
    pref = cfg.get("pref", 2)  # dots group lookahead
    use_prelu = cfg.get("use_prelu", True)  # parametric_relu honors alpha

    ni = r // P  # i-blocks per core (8)
    njt = n // P  # j-tiles (64)
    ng = njt // g  # groups
    nk = IN // P  # k-tiles (4)

    assert ni == 8
    # shared PSUM bank 7 layout (f32 cols): acc7 [0:257], right dots
    # [257:257+njt], left dots [257+njt : 257+njt+ni]
    RD0 = D + 1
    LD0 = RD0 + njt
    assert LD0 + ni <= 512

    nc = bacc.Bacc("TRN2", target_bir_lowering=False, debug=False)

    f8 = mybir.dt.float8e5
    adjm = nc.dram_tensor("adjm", [n, r], f8, kind="ExternalInput")
    ctx_own = nc.dram_tensor("ctx_own", [r, IN], bf16, kind="ExternalInput")
    if cfg.get("dots", "pe") == "dve":
        ctx_dve = nc.dram_tensor("ctx_dve", [n, IN], bf16, kind="ExternalInput")
    ctxT = nc.dram_tensor("ctxT", [IN, n], bf16, kind="ExternalInput")
    rel_in = nc.dram_tensor("rel_in", [n, D + 1], f16, kind="ExternalInput")
    vl_in = nc.dram_tensor("vl_in", [IN], bf16, kind="ExternalInput")
    vr_in = nc.dram_tensor("vr_in", [IN], bf16, kind="ExternalInput")
    lbias = nc.dram_tensor("lbias", [1], f32, kind="ExternalInput")  # b_l+b_r
    out = nc.dram_tensor("out", [r, D], f16, kind="ExternalOutput")
    l_scr = nc.dram_tensor("l_scr", [r], f16)  # left bounce scratch
    use_ag = cfg.get("use_ag", False)
    if use_ag:
        r_shard = nc.dram_tensor("r_shard", [r], f32)
        r_all = nc.dram_tensor("r_all", [n], f32, addr_space="Shared")

    with tile.TileContext(nc) as tc, ExitStack() as ctx:
        singles = ctx.enter_context(tc.tile_pool(name="singles", bufs=1))
        ctxT_pool = ctx.enter_context(tc.tile_pool(name="ctxTp", bufs=2))
        adj_pool = ctx.enter_context(tc.tile_pool(name="adjp", bufs=cfg.get("adj_bufs", 3)))
        u_pool = ctx.enter_context(tc.tile_pool(name="up", bufs=3))
        e1_pool = ctx.enter_context(tc.tile_pool(name="e1p", bufs=cfg.get("e_bufs", 4)))
        e2_pool = ctx.enter_context(tc.tile_pool(name="e2p", bufs=cfg.get("e_bufs", 4)))
        zm_pool = ctx.enter_context(tc.tile_pool(name="zmp", bufs=3))
        out_pool = ctx.enter_context(tc.tile_pool(name="outp", bufs=2))
        sm_pool = ctx.enter_context(tc.tile_pool(name="smp", bufs=2))
        acc_psum = ctx.enter_context(
            tc.tile_pool(name="accps", bufs=1, space="PSUM")
        )
        scr_pool = ctx.enter_context(tc.tile_pool(name="scrp", bufs=1))

        def _emit_body():
            # group spec: (start j-tile, width); smaller groups at the head
            # (faster ramp) and tail (shorter pipeline drain)
            gspecs = []
            pos = 0
            head = cfg.get("head_split", [2, 2])
            tail = cfg.get("tail_split", [2, 1, 1])
            for w in head:
                gspecs.append((pos, w)); pos += w
            while pos < njt - sum(tail):
                gspecs.append((pos, g)); pos += g
            for w in tail:
                gspecs.append((pos, w)); pos += w
            assert pos == njt
            ngr = len(gspecs)

            adj_tiles = {}

            adj_eng = nc.scalar if cfg.get("adj_queue", "act") == "act" else nc.sync

            def emit_adjm_dma(gi):
                # adjm DRAM layout is [p, jt, i] so each partition's group
                # slab is one contiguous run (1 DMA descriptor/partition)
                j0, gw = gspecs[gi]
                adjt = adj_pool.tile([P, gw, r], f8, tag="adj")
                adj_eng.dma_start(
                    out=adjt,
                    in_=bass.AP(
                        tensor=adjm,
                        offset=j0 * r,
                        ap=[[njt * r, P], [r, gw], [1, r]],
                    ),
                )
                adj_tiles[gi] = (adjt, gw)

            adj_pref = cfg.get("adj_pref", 2)
            for _gi in range(min(adj_pref, ngr)):
                emit_adjm_dma(_gi)

            # ---------------- phase 0: params ----------------
            vlb = singles.tile([P, nk], bf16, tag="vlb")
            nc.sync.dma_start(
                out=vlb, in_=bass.AP(tensor=vl_in, offset=0, ap=[[1, P], [P, nk]])
            )
            vrb = singles.tile([P, nk], bf16, tag="vrb")
            nc.sync.dma_start(
                out=vrb, in_=bass.AP(tensor=vr_in, offset=0, ap=[[1, P], [P, nk]])
            )
            lbb = singles.tile([P, 1], f32, tag="lbb")
            nc.sync.dma_start(
                out=lbb, in_=bass.AP(tensor=lbias, offset=0, ap=[[0, P], [1, 1]])
            )

            # relation chunks, each augmented with a ones column
            # (denominator trick).  Per-chunk tiles + deferred DMA keep the
            # big rel read off the critical path and the deps fine-grained.
            nch = cfg.get("dot_chunks", 8)
            tpc = njt // nch  # j-tiles per chunk
            rel_chunks = [
                singles.tile([P, tpc, D + 1], f16, name=f"relch{c}", tag=f"relch{c}")
                for c in range(nch)
            ]

            rel_eng = {"sp": nc.sync, "act": nc.scalar, "pool": nc.gpsimd}[
                cfg.get("rel_queue", "act")
            ]

            def emit_rel_dma(c):
                # rel ships host-packed [p, jt, D+1] with the ones column
                # baked in: one contiguous descriptor per partition
                rel_eng.dma_start(
                    out=rel_chunks[c],
                    in_=bass.AP(
                        tensor=rel_in,
                        offset=c * tpc * (D + 1),
                        ap=[[njt * (D + 1), P], [1, tpc * (D + 1)]],
                    ),
                )

            def rel_tile(jt):
                return rel_chunks[jt // tpc][:, jt % tpc, :]

            # PSUM accumulators: banks 0-6 own i-blocks 0-6; bank 7 shared
            # between acc7 and the left/right dot-product columns.
            accs = [
                acc_psum.tile([P, 512], f32, tag=f"acc{ib}", name=f"acc{ib}")
                for ib in range(ni - 1)
            ]
            shared = acc_psum.tile([P, 512], f32, tag="accsh", name="accsh")
            accs.append(shared)

            # ---------------- dot products (prologue) ----------------
            # left/right dots accumulate in the spare region of shared PSUM
            # bank 7 and are evacuated to SBUF in chunks.  Each chunk gets
            # its OWN SBUF tile: dependency tracking is tile-granular, so a
            # single shared tile would make the first main-loop read wait
            # for the LAST chunk's copy.
            left_sb = singles.tile([P, ni], f32, tag="left_sb")
            left_cols = left_sb[:, :]
            right_chunks = [
                singles.tile([P, tpc], f32, name=f"rchunk{c}", tag=f"rchunk{c}")
                for c in range(nch)
            ]

            def right_col(jt):
                return right_chunks[jt // tpc][:, jt % tpc : jt % tpc + 1]


            # left dots on DVE (STT accum) from natural-layout ctx_own:
            # DVE is idle at program start and this keeps the left_b chain
            # (lc16 -> DRAM bounce -> broadcast), which gates the first
            # u-add, off the PE/prologue critical path.
            vlb_f = singles.tile([P, IN], bf16, tag="vlb_f")
            nc.sync.dma_start(
                out=vlb_f,
                in_=bass.AP(tensor=vl_in, offset=0, ap=[[0, P], [1, IN]]),
            )
            cow = singles.tile([P, ni, IN], bf16, tag="cow")
            nc.sync.dma_start(
                out=cow,
                in_=bass.AP(
                    tensor=ctx_own, offset=0, ap=[[ni * IN, P], [1, ni * IN]]
                ),
            )
            for t in range(ni):
                scr = scr_pool.tile([P, IN], f32, tag="scr")
                nc.vector.scalar_tensor_tensor(
                    out=scr, in0=cow[:, t, :], scalar=0.0, in1=vlb_f,
                    op0=mybir.AluOpType.bypass, op1=mybir.AluOpType.mult,
                    accum_out=left_cols[:, t : t + 1],
                )

            if use_ag:
                # right dots for OWN rows only (from cow, like left), then
                # AllGather the 8192-float right vector (32KB) instead of
                # re-reading the full replicated ctxT (8.4MB per core)
                vrb_f = singles.tile([P, IN], bf16, tag="vrb_f")
                nc.sync.dma_start(
                    out=vrb_f,
                    in_=bass.AP(tensor=vr_in, offset=0, ap=[[0, P], [1, IN]]),
                )
                r_own = singles.tile([P, ni], f32, tag="r_own")
                for t in range(ni):
                    scr = scr_pool.tile([P, IN], f32, tag="scr")
                    nc.vector.scalar_tensor_tensor(
                        out=scr, in0=cow[:, t, :], scalar=0.0, in1=vrb_f,
                        op0=mybir.AluOpType.bypass, op1=mybir.AluOpType.mult,
                        accum_out=r_own[:, t : t + 1],
                    )
                nc.sync.dma_start(
                    out=bass.AP(tensor=r_shard, offset=0, ap=[[1, P], [P, ni]]),
                    in_=r_own,
                )
                nc.gpsimd.collective_compute(
                    "AllGather",
                    mybir.AluOpType.bypass,
                    replica_groups=[list(range(NCORES))],
                    ins=[r_shard[:]],
                    outs=[r_all[:]],
                )
                for c in range(nch):
                    nc.sync.dma_start(
                        out=right_chunks[c],
                        in_=bass.AP(
                            tensor=r_all,
                            offset=c * tpc * P,
                            ap=[[1, P], [P, tpc]],
                        ),
                    )
                    emit_rel_dma(c)
            elif dots == "pe":
                first_mm = [True]
                # right dots in chunks, each evacuated to SBUF as soon as
                # ready so the main loop starts after chunk 0 (not all 64)
                for ch_i in range(nch):
                    for tt in range(tpc):
                        jt = ch_i * tpc + tt
                        if tt == 0:
                            # ctxT DRAM layout is [p, chunk, kt, j']: one
                            # contiguous nk*tpc*P run per partition
                            cht = ctxT_pool.tile(
                                [P, nk, tpc * P], bf16, tag="ctxT"
                            )
                            nc.sync.dma_start(
                                out=cht,
                                in_=bass.AP(
                                    tensor=ctxT,
                                    offset=ch_i * nk * tpc * P,
                                    ap=[
                                        [nch * nk * tpc * P, P],
                                        [1, nk * tpc * P],
                                    ],
                                ),
                            )
                        for k in range(nk):
                            nc.tensor.matmul(
                                shared[:, RD0 + jt : RD0 + jt + 1],
                                lhsT=cht[:, k, tt * P : (tt + 1) * P],
                                rhs=vrb[:, k : k + 1],
                                start=first_mm[0],  # zeroes bank 7 once/rep
                                stop=False,
                                skip_group_check=True,
                            )
                            first_mm[0] = False
                    nc.vector.tensor_copy(
                        right_chunks[ch_i],
                        shared[:, RD0 + ch_i * tpc : RD0 + (ch_i + 1) * tpc],
                    )
                    emit_rel_dma(ch_i)
            else:
                # DVE STT dots from packed ctx_dve; acc bank 7 is then a
                # normal accumulator (no shared-bank trickery at all)
                vrb_f = singles.tile([P, IN], bf16, tag="vrb_f")
                nc.sync.dma_start(
                    out=vrb_f,
                    in_=bass.AP(tensor=vr_in, offset=0, ap=[[0, P], [1, IN]]),
                )
                for ch_i in range(nch):
                    cdt = ctxT_pool.tile([P, tpc, IN], bf16, tag="ctxT")
                    nc.sync.dma_start(
                        out=cdt,
                        in_=bass.AP(
                            tensor=ctx_dve,
                            offset=ch_i * tpc * IN,
                            ap=[[njt * IN, P], [1, tpc * IN]],
                        ),
                    )
                    for tt in range(tpc):
                        jt = ch_i * tpc + tt
                        scr = scr_pool.tile([P, IN], f32, tag="scr")
                        nc.vector.scalar_tensor_tensor(
                            out=scr, in0=cdt[:, tt, :], scalar=0.0, in1=vrb_f,
                            op0=mybir.AluOpType.bypass,
                            op1=mybir.AluOpType.mult,
                            accum_out=right_col(jt),
                        )
                    emit_rel_dma(ch_i)

            # left + (b_l + b_r) -> fp16, bounce via DRAM, broadcast back
            lc16 = singles.tile([P, ni], f16, tag="lc16")
            nc.vector.tensor_scalar(
                out=lc16, in0=left_cols, scalar1=lbb[:, 0:1], scalar2=None,
                op0=mybir.AluOpType.add,
            )
            # bounce + broadcast ride the Pool SWDGE queue: their sem waits
            # must not block the ACT sequencer (exps) or SP queue (bulk DMA)
            bq = cfg.get("bounce_queue", "act")
    
            bounce_eng = {"pool": nc.gpsimd, "act": nc.scalar, "sp": nc.sync}[bq]
            bounce_eng.dma_start(
                out=bass.AP(tensor=l_scr, offset=0, ap=[[1, P], [P, ni]]),
                in_=lc16,
            )
            left_b = singles.tile([P, g, r], f16, tag="left_b")
            for t in range(g):
                bounce_eng.dma_start(
                    out=left_b[:, t, :],
                    in_=bass.AP(tensor=l_scr, offset=0, ap=[[0, P], [1, r]]),
                )

            # ------------------------- main loop ----------------------------
            # which groups use the 1-exp variant / pool u-add (spread evenly)
            n_1exp_done = 0
            n_pool_done = 0
            pending = []  # software pipeline: consume stage delayed 1 group

            def consume(ent):
                # 2-exp groups: the DVE max is emitted here, one iteration
                # after its exps, so DVE's in-order queue never stalls on ACT
                j0, gw, kind, tiles = ent
                if kind == "2exp":
                    e1t, e2t, zmt = tiles
                    nc.vector.tensor_max(zmt, e1t, e2t)
                else:
                    zmt = tiles[0]
                for tt in range(gw):
                    jt = j0 + tt
                    for ib in range(ni):
                        nc.tensor.matmul(
                            accs[ib][:, 0 : D + 1],
                            lhsT=zmt[:, tt, ib * P : (ib + 1) * P],
                            rhs=rel_tile(jt),
                            start=(jt == 0 and (ib < ni - 1 or dots != "pe")),
                            stop=(jt == njt - 1),
                            skip_group_check=(ib == ni - 1 and dots == "pe"),
                        )

            for gi in range(ngr):
                if gi + adj_pref < ngr:
                    emit_adjm_dma(gi + adj_pref)
                adjt, gw = adj_tiles.pop(gi)
                j0 = gspecs[gi][0]
                # u = (adjm + right_j) + left_i fused in one STT pass
                ut = u_pool.tile([P, gw, r], f16, tag="u")
                for tt in range(gw):
                    jt = j0 + tt
                    nc.vector.scalar_tensor_tensor(
                        out=ut[:, tt, :], in0=adjt[:, tt, :],
                        scalar=right_col(jt), in1=left_b[:, tt, :],
                        op0=mybir.AluOpType.add, op1=mybir.AluOpType.add,
                    )

                use_1exp = (n1exp * (gi + 1)) // ngr > n_1exp_done
                zmt = zm_pool.tile([P, gw, r], f16, tag="zm")
                if use_1exp:
                    # DVE-heavy: leaky via TS(0.2u) + max, single ACT exp
                    n_1exp_done += 1
                    at = e1_pool.tile([P, gw, r], f16, tag="e1")
                    nc.vector.tensor_scalar(
                        out=at, in0=ut, scalar1=0.2, scalar2=None,
                        op0=mybir.AluOpType.mult,
                    )
                    tt_ = e2_pool.tile([P, gw, r], f16, tag="e2")
                    nc.vector.tensor_max(tt_, ut, at)
                    nc.scalar.activation(
                        zmt, tt_, mybir.ActivationFunctionType.Exp,
                        bias=0.0, scale=1.0,
                    )
                    pending.append((j0, gw, "1exp", (zmt,)))
                elif use_prelu:
                    # ACT-only: parametric_relu (alpha=0.2) + exp, both in
                    # the exp table set -> no table reload, no DVE max
                    t16 = e1_pool.tile([P, gw, r], f16, tag="e1")
                    nc.scalar.activation(
                        t16, ut, mybir.ActivationFunctionType.Prelu,
                        bias=0.0, scale=1.0, alpha=0.2,
                    )
                    nc.scalar.activation(
                        zmt, t16, mybir.ActivationFunctionType.Exp,
                        bias=0.0, scale=1.0,
                    )
                    pending.append((j0, gw, "1exp", (zmt,)))
                else:
                    e1t = e1_pool.tile([P, gw, r], f16, tag="e1")
                    nc.scalar.activation(
                        e1t, ut, mybir.ActivationFunctionType.Exp,
                        bias=0.0, scale=1.0,
                    )
                    e2t = e2_pool.tile([P, gw, r], f16, tag="e2")
                    nc.scalar.activation(
                        e2t, ut, mybir.ActivationFunctionType.Exp,
                        bias=0.0, scale=0.2,
                    )
                    pending.append((j0, gw, "2exp", (e1t, e2t, zmt)))

                if len(pending) > 1:
                    consume(pending.pop(0))
            while pending:
                consume(pending.pop(0))

            # ------------------------ finalize ------------------------------
            # finalize entirely on DVE: relu(num/den) = max(num*recip, 0)
            # (recip > 0), avoiding ACT table swaps and engine ping-pong
            for ib in range(ni):
                recip = sm_pool.tile([P, 1], f32, tag="recip")
                nc.vector.reciprocal(recip, accs[ib][:, D : D + 1])
                ob = out_pool.tile([P, D], f16, tag="ob")
                nc.vector.tensor_scalar(
                    out=ob, in0=accs[ib][:, 0:D], scalar1=recip[:, 0:1],
                    scalar2=0.0, op0=mybir.AluOpType.mult,
                    op1=mybir.AluOpType.max,
                )
                nc.sync.dma_start(out=out[ib * P : (ib + 1) * P, :], in_=ob)

        flat_reps = cfg.get("flat_reps", 1)  # sim-only: unrolled reps
        if reps > 1:
            with tc.For_i(0, reps, 1):
                _emit_body()
        else:
            for _ in range(flat_reps):
                _emit_body()

    nc.compile()
    return nc


_BASE_CFG = dict(n=N, r=N // NCORES, g=4, n1exp=10, pool_add=0, dots="pe",
                 pref=2, use_prelu=True, dot_chunks=8, head_split=[2, 2],
                 tail_split=[2, 1, 1], e_bufs=4)


def _get_program(cfg_key):
    if cfg_key not in _CACHE:
        _CACHE[cfg_key] = build_program(dict(_BASE_CFG))
    return _CACHE[cfg_key]


def prepare_in_maps(relation, context, adj_tensor, W_common, w_left, b_left,
                    w_right, b_right):
    relation = np.asarray(relation, dtype=np.float32)
    context = np.asarray(context, dtype=np.float32)
    adj_tensor = np.asarray(adj_tensor, dtype=np.float32)
    W_common = np.asarray(W_common, dtype=np.float32)
    w_left = np.asarray(w_left, dtype=np.float32)
    w_right = np.asarray(w_right, dtype=np.float32)
    b_l = float(np.asarray(b_left))
    b_r = float(np.asarray(b_right))

    # host-side parameter folding (weights only, no activations)
    v_left = (W_common.T @ w_left).astype(ml_dtypes.bfloat16)
    v_right = (W_common.T @ w_right).astype(ml_dtypes.bfloat16)
    lbias = np.array([b_l + b_r], dtype=np.float32)

    relh = np.concatenate(
        [relation, np.ones((N, 1), np.float32)], axis=1
    ).astype(np.float16)
    relh = np.ascontiguousarray(
        relh.reshape(N // P, P, D + 1).transpose(1, 0, 2)
    )  # [p, jt, D+1]
    ctxb = context.astype(ml_dtypes.bfloat16)
    # ctxT packed as [p, chunk, kt, j']: partition-contiguous chunk reads
    nch = _BASE_CFG.get("dot_chunks", 8)
    tpcP = N // nch  # j per chunk
    ctxT8 = np.ascontiguousarray(
        ctxb.T.reshape(4, P, nch, tpcP).transpose(1, 2, 0, 3)
    )  # [128, nch, 4, tpcP]

    # adjm[j, i] = MASKB * (adj[i, j] - 1):  0 if edge, -512 if masked
    adjm_full = ((adj_tensor - 1.0) * MASKB).astype(ml_dtypes.float8_e5m2)

    rows = N // NCORES
    njt = N // P
    in_maps = []
    for c in range(NCORES):
        sl = slice(c * rows, (c + 1) * rows)
        # adjm slab [j, i] -> [p, jt, i] (partition-contiguous group reads)
        aT = adjm_full[sl].T.reshape(njt, P, rows)
        m = {
            "adjm": np.ascontiguousarray(aT.transpose(1, 0, 2)),
            "ctx_own": np.ascontiguousarray(
                ctxb[sl].reshape(rows // P, P, IN).transpose(1, 0, 2)
            ),
            "ctxT": ctxT8,
            "rel_in": relh,
            "vl_in": v_left,
            "vr_in": v_right,
            "lbias": lbias,
        }
        in_maps.append(m)
    return in_maps


# ------------------------------------------------------------------- entry
def kernel(relation, context, adj_tensor, W_common, w_left, b_left, w_right,
           b_right):
    from concourse.bass_utils import run_bass_kernel_spmd

    in_maps = prepare_in_maps(relation, context, adj_tensor, W_common,
                              w_left, b_left, w_right, b_right)
    nc = _get_program("main")
    last_err = None
    for _attempt in range(3):
        try:
            res = run_bass_kernel_spmd(nc, in_maps, list(range(NCORES)))
            outs = [np.asarray(res.results[c]["out"]) for c in range(NCORES)]
            return np.concatenate(outs, axis=0).astype(np.float32)
        except Exception as e:  # transient device-unrecoverable seen on axon
            last_err = e
            import time as _time

            try:
                import jax

                jax.clear_caches()
            except Exception:
                pass
            _time.sleep(3.0)
    raise last_err
